# revision 1
# baseline (speedup 1.0000x reference)
"""CDD loss kernel for 8 Trainium2 NeuronCores (Bass/Tile, SPMD).

Math (validated vs reference in float32):
  ps is one-hot -> every (C,C,N,N) reference tensor collapses to per-class-
  block sums. Host sorts+pads src rows by class (CAP rows/class, pads are
  huge distinct sentinel vectors so exp(-dist/bw) underflows to exactly 0).
  The E_pp class-diagonal blocks have their diagonal zeroed on device, making
  each diagonal entry contribute exactly exp(0)=1 per bandwidth; the exact
  correction (5*CAP - 5*exp(-1e-5)*cs) is applied as a host-computed offset.
  g2 is symmetric -> T2 = T1^T, so inter = sum_{s!=t} 2*(T1-T3)/(C^2-C).

Distribution (SPMD, one program, per-core data):
  - every core computes E rows for its class pair (rotation of the padded
    src rows makes "own" rows/cols sit at fixed offsets), partial
    S1 = Wown^T E_pp W, sst = Wown^T E_pt pt, stt = pt^T E_tt pt / 8
  - one AllReduce of the packed [12,36] partials
  - gammas + negative inverse bandwidths on device (tiny DVE ops)
  - exp-heavy sums: T1/k1 and T3 run as single ACT instructions over
    flattened broadcast tiles with per-partition scale and accum_out
    (free-dim reduction inside the ACT op); k2/k3 as [128,*] passes
  - per-core weighted reduce with host weight matrix -> [intra, inter]
    partials, host sums the 8 partials.
"""

import math
import numpy as np

C = 12
KN = 5
MU = 2
N = 384
D = 256
CAP = 64
R = C * CAP            # 768 padded src rows
NCORES = 8
NCOL = 14              # ACC columns: T1, T3, k1*5, k3*5, k2*2
DIAG5 = 5.0 * math.exp(-1e-5)
I2 = 2.0 / (C * C - C)

_COMPILED = {}


# ----------------------------------------------------------------------------
# host-side prep
# ----------------------------------------------------------------------------

def _host_prep(src_x, tgt_x, src_y, tgt_y):
    src_x = np.ascontiguousarray(np.asarray(src_x, dtype=np.float32))
    tgt_x = np.ascontiguousarray(np.asarray(tgt_x, dtype=np.float32))
    src_y = np.asarray(src_y).astype(np.int64)
    pt = np.ascontiguousarray(np.asarray(tgt_y, dtype=np.float32))

    counts = np.bincount(src_y, minlength=C)
    if counts.max() > CAP:
        return None  # caller falls back to numpy path

    perm = np.argsort(src_y, kind="stable")
    sx_pad = np.zeros((R, D), np.float32)
    W = np.zeros((R, C), np.float32)
    # pad sentinels: huge random-sign vectors. Pad-pad dot products are then
    # tiny relative to the norms (no catastrophic cancellation in d2), every
    # pad-involved distance is >= ~3e5 and exp(-dist/bw) underflows to 0.
    rng = np.random.default_rng(987654321)
    sgn = (rng.integers(0, 2, size=(R, D)).astype(np.float32) * 2.0 - 1.0)
    off = 0
    padidx = 0
    for c in range(C):
        idx = perm[off:off + counts[c]]
        sx_pad[c * CAP:c * CAP + counts[c]] = src_x[idx]
        W[c * CAP:c * CAP + counts[c], c] = 1.0
        for p in range(CAP - counts[c]):
            sx_pad[c * CAP + counts[c] + p, :] = 2.0e4 * sgn[padidx]
            padidx += 1
        off += counts[c]

    cs = counts.astype(np.float64)
    ct = pt.sum(0).astype(np.float64)
    pss = cs * cs
    ptt = ct * ct

    rden2 = (1.0 / (pss[:, None] + pss[None, :]
                    + 2.0 * cs[:, None] * cs[None, :])).astype(np.float32)
    rdenin = (1.0 / (pss + ptt + 2.0 * cs * ct)).astype(np.float32).reshape(C, 1)

    eye128 = np.eye(128, dtype=np.float32)
    diagm = np.concatenate([1.0 - np.eye(CAP, dtype=np.float32)] * 2, axis=0)
    eye12 = np.eye(C, dtype=np.float32)
    pw60 = np.zeros((C, 60), np.float32)
    for k in range(KN):
        pw60[:, k * 12:(k + 1) * 12] = -(float(MU) ** (k - KN // 2))
    pw5 = np.zeros((C, 5), np.float32)
    for k in range(KN):
        pw5[:, k] = -(float(MU) ** (k - KN // 2))
    ones128 = np.ones((128, 1), np.float32)
    ssel = np.zeros((NCOL, 2), np.float32)
    ssel[2:14, 0] = 1.0   # intra cols: k1 (2-6), k3 (7-11), k2 (12-13)
    ssel[0:2, 1] = 1.0    # inter cols: T1, T3

    in_maps = []
    for r in range(NCORES):
        g = r % 6
        a, b = 2 * g, 2 * g + 1
        pp_active = r < 6
        roll = 2 * g * CAP

        sxf = np.ascontiguousarray(np.roll(sx_pad, -roll, axis=0))
        wr = np.ascontiguousarray(np.roll(W, -roll, axis=0))
        wown = wr[0:128].copy() if pp_active else np.zeros((128, C), np.float32)

        oh2 = np.zeros((C, 2), np.float32)
        oh2[a, 0] = 1.0
        oh2[b, 1] = 1.0

        k2cls = []
        for q in range(2):
            c = r + 8 * q
            k2cls.append(c if c < C else -1)
        k2sel = np.zeros((C, 2), np.float32)
        ptrow2 = np.zeros((2, N), np.float32)
        ptcolf = np.zeros((128, 6), np.float32)
        for q, c in enumerate(k2cls):
            cc = c if c >= 0 else 0
            k2sel[cc, q] = 1.0
            ptrow2[q] = pt[:, cc]
            for blk in range(3):
                ptcolf[:, q * 3 + blk] = pt[blk * 128:(blk + 1) * 128, cc]

        ptr3a = pt[:, a].reshape(1, N).astype(np.float32)
        ptr3b = pt[:, b].reshape(1, N).astype(np.float32)

        # reindex matrix for the T3 scale column:
        # dest t*5+k <- source k*12 + rot(t) with rot(t) = (2g+t) % 12
        perm65 = np.zeros((65, 65), np.float32)
        for t in range(12):
            for k in range(KN):
                perm65[k * 12 + ((2 * g + t) % 12), t * 5 + k] = 1.0
        for j in range(60, 65):
            perm65[j, j] = 1.0

        wm = np.zeros((128, NCOL), np.float32)
        if pp_active:
            for h, cls in ((0, a), (1, b)):
                for k in range(KN):
                    for t in range(12):
                        if t != cls:
                            wm[h * 64 + k * 12 + t, 0] = I2 / pss[cls]
                for t in range(12):
                    rt_ = (2 * g + t) % 12
                    if rt_ != cls:
                        for k in range(KN):
                            wm[h * 64 + t * 5 + k, 1] = -I2 / (cs[cls] * cs[rt_])
                for k in range(KN):
                    wm[h * CAP:(h + 1) * CAP, 2 + k] = 1.0 / (C * pss[cls])
                    wm[h * CAP:(h + 1) * CAP, 7 + k] = -2.0 / (C * cs[cls] * ct[cls])
        for q, c in enumerate(k2cls):
            if c >= 0:
                wm[:, 12 + q] = 1.0 / (C * ptt[c])

        offs = np.zeros((1, 2), np.float32)
        if r == 0:
            corr = 5.0 * CAP - DIAG5 * cs
            offs[0, 0] = -(corr / pss / C).sum()
            offs[0, 1] = -((C - 1) * corr * I2 / pss).sum()

        in_maps.append({
            "sxf": sxf, "tx": tgt_x, "pt": pt, "wr": wr, "wown": wown,
            "eye128": eye128, "diagm": diagm, "eye12": eye12,
            "oh2": oh2, "k2sel": k2sel, "perm65": perm65,
            "pw60": pw60, "pw5": pw5, "rden2": rden2, "rdenin": rdenin,
            "wm": wm.astype(np.float32), "ssel": ssel,
            "offs": offs, "ones128": ones128,
            "ptr2a": ptrow2[0:1].copy(), "ptr2b": ptrow2[1:2].copy(),
            "ptr3a": ptr3a, "ptr3b": ptr3b, "ptcolf": ptcolf,
        })
    return in_maps


def _numpy_fallback(src_x, tgt_x, src_y, tgt_y):
    f = np.float32
    src_x = np.asarray(src_x, f)
    tgt_x = np.asarray(tgt_x, f)
    src_y = np.asarray(src_y).astype(np.int64)
    pt = np.asarray(tgt_y, f)
    ps = np.eye(C, dtype=f)[src_y]

    def cdist(a, bb):
        d2 = (a * a).sum(1)[:, None] + (bb * bb).sum(1)[None, :] - 2.0 * (a @ bb.T)
        return np.sqrt(np.maximum(d2, 0.0))

    def kern(dist, g):
        acc = 0.0
        for i in range(KN):
            bw = np.maximum(np.asarray(g) * (MU ** (i - KN // 2)), 1e-5)
            acc = acc + np.exp(-np.clip(dist / bw, 1e-5, 1e5))
        return acc

    E_ss = cdist(src_x, src_x); E_tt = cdist(tgt_x, tgt_x); E_st = cdist(src_x, tgt_x)
    sss = np.einsum('ic,ij,jc->c', ps, E_ss, ps)
    stt = np.einsum('ic,ij,jc->c', pt, E_tt, pt)
    sst = np.einsum('is,ij,jt->st', ps, E_st, pt)
    cs = ps.sum(0); ct = pt.sum(0)
    pss = cs * cs; ptt = ct * ct; pstd = cs * ct
    g_in = (sss + stt + 2 * np.diagonal(sst)) / (pss + ptt + 2 * pstd)
    Pss = ps.T[:, :, None] * ps.T[:, None, :]
    Ptt = pt.T[:, :, None] * pt.T[:, None, :]
    Pst = ps.T[:, :, None] * pt.T[:, None, :]
    k1 = (kern(E_ss[None] * Pss, g_in[:, None, None]) * Pss).sum((-2, -1)) / pss
    k2 = (kern(E_tt[None] * Ptt, g_in[:, None, None]) * Ptt).sum((-2, -1)) / ptt
    k3 = (kern(E_st[None] * Pst, g_in[:, None, None]) * Pst).sum((-2, -1)) / pstd
    intra = (k1 + k2 - 2 * k3).sum() / C
    sst_s = np.einsum('is,ij,jt->st', ps, E_ss, ps)
    g2 = (sss[:, None] + sss[None, :] + 2 * sst_s) / (
        pss[:, None] + pss[None, :] + 2 * cs[:, None] * cs[None, :])
    T1 = np.zeros((C, C), f); T3 = np.zeros((C, C), f)
    for s in range(C):
        ms = ps[:, s].astype(bool)
        for t in range(C):
            mt = ps[:, t].astype(bool)
            T1[s, t] = kern(E_ss[np.ix_(ms, ms)], g2[s, t]).sum() / pss[s]
            T3[s, t] = kern(E_ss[np.ix_(ms, mt)], g2[s, t]).sum() / (cs[s] * cs[t])
    inter = ((2 * T1 - 2 * T3) * (1 - np.eye(C))).sum() / (C * C - C)
    return np.array([intra, inter], np.float32)


# ----------------------------------------------------------------------------
# device program
# ----------------------------------------------------------------------------

def _build_program():
    import os
    import concourse.bass as bass
    import concourse.tile as tile
    from concourse import bacc, mybir

    STAGE = int(os.environ.get("CDD_STAGE", "99"))

    f32 = mybir.dt.float32
    AF = mybir.ActivationFunctionType
    OP = mybir.AluOpType

    nc = bacc.Bacc("TRN2", target_bir_lowering=False, debug=False,
                   num_devices=NCORES)

    def din(name, shape):
        return nc.dram_tensor(name, list(shape), f32, kind="ExternalInput").ap()

    i_sxf = din("sxf", (R, D))
    i_tx = din("tx", (N, D))
    i_pt = din("pt", (N, C))
    i_wr = din("wr", (R, C))
    i_wown = din("wown", (128, C))
    i_eye128 = din("eye128", (128, 128))
    i_diagm = din("diagm", (128, CAP))
    i_eye12 = din("eye12", (C, C))
    i_oh2 = din("oh2", (C, 2))
    i_k2sel = din("k2sel", (C, 2))
    i_perm65 = din("perm65", (65, 65))
    i_pw60 = din("pw60", (C, 60))
    i_pw5 = din("pw5", (C, 5))
    i_rden2 = din("rden2", (C, C))
    i_rdenin = din("rdenin", (C, 1))
    i_wm = din("wm", (128, NCOL))
    i_ssel = din("ssel", (NCOL, 2))
    i_offs = din("offs", (1, 2))
    i_ones = din("ones128", (128, 1))
    i_ptr2a = din("ptr2a", (1, N))
    i_ptr2b = din("ptr2b", (1, N))
    i_ptr3a = din("ptr3a", (1, N))
    i_ptr3b = din("ptr3b", (1, N))
    i_ptcolf = din("ptcolf", (128, 6))

    o_out = nc.dram_tensor("out", [1, 2], f32, kind="ExternalOutput").ap()
    o_sred = nc.dram_tensor("dbg_sred", [C, 36], f32, kind="ExternalOutput").ap()
    o_acc = nc.dram_tensor("dbg_acc", [128, NCOL], f32, kind="ExternalOutput").ap()
    o_g2 = nc.dram_tensor("dbg_g2", [C, C], f32, kind="ExternalOutput").ap()
    o_ibg = nc.dram_tensor("dbg_ibg", [C, 65], f32, kind="ExternalOutput").ap()

    with tile.TileContext(nc) as tc:
        with (
            tc.tile_pool(name="io", bufs=1) as io,
            tc.tile_pool(name="big", bufs=1) as big,
            tc.tile_pool(name="scr", bufs=2) as scr,
            tc.tile_pool(name="sm", bufs=1) as sm,
            tc.tile_pool(name="pG", bufs=2, space="PSUM") as pG,
            tc.tile_pool(name="pA", bufs=1, space="PSUM") as pA,
            tc.tile_pool(name="pT", bufs=2, space="PSUM") as pT,
            tc.tile_pool(name="pS", bufs=1, space="PSUM") as pS,
            tc.tile_pool(name="dram", bufs=1, space="DRAM") as dpool,
        ):
            dma = nc.sync.dma_start

            def load(name, ap_in, shape):
                t = io.tile(list(shape), f32, tag=name, name=name)
                dma(out=t[:], in_=ap_in[:])
                return t

            sxf = [load(f"sxf{i}", i_sxf[i * 128:(i + 1) * 128, :], (128, D))
                   for i in range(6)]
            tx = [load(f"tx{i}", i_tx[i * 128:(i + 1) * 128, :], (128, D))
                  for i in range(3)]
            ptb = [load(f"pt{i}", i_pt[i * 128:(i + 1) * 128, :], (128, C))
                   for i in range(3)]
            wrb = [load(f"wr{i}", i_wr[i * 128:(i + 1) * 128, :], (128, C))
                   for i in range(6)]
            wown = load("wown", i_wown, (128, C))
            eye128 = load("eye128", i_eye128, (128, 128))
            diagm = load("diagm", i_diagm, (128, CAP))
            eye12 = load("eye12", i_eye12, (C, C))
            oh2 = load("oh2", i_oh2, (C, 2))
            k2sel = load("k2sel", i_k2sel, (C, 2))
            perm65 = load("perm65", i_perm65, (65, 65))
            pw60 = load("pw60", i_pw60, (C, 60))
            pw5 = load("pw5", i_pw5, (C, 5))
            rden2 = load("rden2", i_rden2, (C, C))
            rdenin = load("rdenin", i_rdenin, (C, 1))
            wm = load("wm", i_wm, (128, NCOL))
            ssel = load("ssel", i_ssel, (NCOL, 2))
            offs = load("offs", i_offs, (1, 2))
            ones = load("ones128", i_ones, (128, 1))
            ptr2 = [load("ptr2a", i_ptr2a, (1, N)),
                    load("ptr2b", i_ptr2b, (1, N))]
            ptr3 = [load("ptr3a", i_ptr3a, (1, N)),
                    load("ptr3b", i_ptr3b, (1, N))]
            ptcolf = load("ptcolf", i_ptcolf, (128, 6))

            if STAGE >= 11:
                # ---------------- transposes: sxfT, txT ----------------
                sxfT = [big.tile([128, R], f32, tag=f"sxfT{k}", name=f"sxfT{k}")
                        for k in range(2)]
                txT = [big.tile([128, N], f32, tag=f"txT{k}", name=f"txT{k}")
                       for k in range(2)]
                for m in range(6):
                    for k in range(2):
                        tp_ = pT.tile([128, 128], f32, tag="tiny", name="tp")
                        nc.tensor.transpose(tp_[:], sxf[m][:, k * 128:(k + 1) * 128],
                                            eye128[:])
                        nc.vector.tensor_copy(sxfT[k][:, m * 128:(m + 1) * 128], tp_[:])
                for m in range(3):
                    for k in range(2):
                        tp_ = pT.tile([128, 128], f32, tag="tiny", name="tp")
                        nc.tensor.transpose(tp_[:], tx[m][:, k * 128:(k + 1) * 128],
                                            eye128[:])
                        nc.vector.tensor_copy(txT[k][:, m * 128:(m + 1) * 128], tp_[:])

            if STAGE >= 12:
                # ---------------- row norms ----------------
                rscol = [sm.tile([128, 1], f32, tag=f"rs{m}", name=f"rs{m}")
                         for m in range(6)]
                rtcol = [sm.tile([128, 1], f32, tag=f"rt{m}", name=f"rt{m}")
                         for m in range(3)]
                for m in range(6):
                    nsc = scr.tile([128, D], f32, tag="normscr", name="nsc")
                    nc.scalar.activation(nsc[:], sxf[m][:], AF.Square,
                                         accum_out=rscol[m][:])
                for m in range(3):
                    nsc = scr.tile([128, D], f32, tag="normscr", name="nsc")
                    nc.scalar.activation(nsc[:], tx[m][:], AF.Square,
                                         accum_out=rtcol[m][:])

                rsrow = sm.tile([1, R], f32, tag="rsrow", name="rsrow")
                rtrow = sm.tile([1, N], f32, tag="rtrow", name="rtrow")
                for m in range(6):
                    tp_ = pT.tile([1, 128], f32, tag="tiny", name="tpr")
                    nc.tensor.transpose(tp_[:], rscol[m][:], eye128[:])
                    nc.vector.tensor_copy(rsrow[:, m * 128:(m + 1) * 128], tp_[:])
                for m in range(3):
                    tp_ = pT.tile([1, 128], f32, tag="tiny", name="tpr")
                    nc.tensor.transpose(tp_[:], rtcol[m][:], eye128[:])
                    nc.vector.tensor_copy(rtrow[:, m * 128:(m + 1) * 128], tp_[:])

                rsrowb = big.tile([128, R], f32, tag="rsrowb", name="rsrowb")
                rtrowb = big.tile([128, N], f32, tag="rtrowb", name="rtrowb")
                nc.gpsimd.partition_broadcast(rsrowb[:], rsrow[:])
                nc.gpsimd.partition_broadcast(rtrowb[:], rtrow[:])

            if STAGE >= 13:
                # ---------------- E matrices ----------------
                def emit_E(dst, lhsT_tiles, lhs_lo, rhs_tiles, n_cols, rcol, rowb):
                    done = 0
                    while done < n_cols:
                        nchunk = min(512, n_cols - done)
                        gp = pG.tile([128, 512], f32, tag="G", name="gp")
                        for k in range(2):
                            nc.tensor.matmul(
                                gp[:, :nchunk],
                                lhsT_tiles[k][:, lhs_lo:lhs_lo + 128],
                                rhs_tiles[k][:, done:done + nchunk],
                                start=(k == 0), stop=(k == 1))
                        t1_ = scr.tile([128, 512], f32, tag="d2scr", name="d2s")
                        nc.vector.scalar_tensor_tensor(
                            out=t1_[:, :nchunk], in0=gp[:, :nchunk], scalar=-2.0,
                            in1=rowb[:, done:done + nchunk],
                            op0=OP.mult, op1=OP.add)
                        nc.vector.tensor_scalar(
                            t1_[:, :nchunk], t1_[:, :nchunk],
                            rcol[:], 0.0, OP.add, OP.max)
                        nc.scalar.activation(dst[:, done:done + nchunk],
                                             t1_[:, :nchunk], AF.Sqrt)
                        done += nchunk

                E_own = big.tile([128, R], f32, tag="E_own", name="E_own")
                emit_E(E_own, sxfT, 0, sxfT, R, rscol[0], rsrowb)

                E_ttf = big.tile([128, 3 * N], f32, tag="E_ttf", name="E_ttf")
                for blk in range(3):
                    emit_E(E_ttf[:, blk * N:(blk + 1) * N], txT, blk * 128, txT, N,
                           rtcol[blk], rtrowb)

                E_pt = big.tile([128, N], f32, tag="E_pt", name="E_pt")
                emit_E(E_pt, sxfT, 0, txT, N, rscol[0], rtrowb)

            if STAGE >= 20:
                # diag-zeroed own-class diagonal blocks [128, 64]
                E_diag = big.tile([128, CAP], f32, tag="E_diag", name="E_diag")
                nc.vector.tensor_tensor(E_diag[0:CAP, :], E_own[0:CAP, 0:CAP],
                                        diagm[0:CAP, :], OP.mult)
                nc.vector.tensor_tensor(E_diag[CAP:128, :],
                                        E_own[CAP:128, CAP:128],
                                        diagm[CAP:128, :], OP.mult)

                # E -> DRAM for the flat broadcast reads
                d_eo = dpool.tile([128, R], f32, tag="d_eo", name="d_eo")
                d_ed = dpool.tile([128, CAP], f32, tag="d_ed", name="d_ed")
                dma(out=d_eo[:], in_=E_own[:])
                dma(out=d_ed[:], in_=E_diag[:])

                t1src = big.tile([128, CAP * CAP], f32, tag="t1src", name="t1src")
                for h in range(2):
                    ap_in = bass.AP(tensor=d_ed.tensor, offset=h * CAP * CAP,
                                    ap=[[0, 64], [1, CAP * CAP]])
                    dma(out=t1src[h * 64:(h + 1) * 64, :], in_=ap_in)
                t3src = big.tile([128, CAP * CAP], f32, tag="t3src", name="t3src")
                nc.vector.memset(t3src[:], 0.0)
                for h in range(2):
                    for t in range(12):
                        ap_in = bass.AP(tensor=d_eo.tensor,
                                        offset=h * CAP * R + t * CAP,
                                        ap=[[0, 5], [R, CAP], [1, CAP]])
                        p0 = h * 64 + t * 5
                        dma(out=t3src[p0:p0 + 5, :], in_=ap_in)

                # ---------------- k2 / k3 static builds ----------------
                ptrow2b = [big.tile([128, N], f32, tag=f"ptrow2b{q}",
                                    name=f"ptrow2b{q}") for q in range(2)]
                nc.gpsimd.partition_broadcast(ptrow2b[0][:], ptr2[0][:])
                nc.gpsimd.partition_broadcast(ptrow2b[1][:], ptr2[1][:])
                ptw3 = big.tile([128, N], f32, tag="ptw3", name="ptw3")
                ptw3t = big.tile([128, N], f32, tag="ptw3t", name="ptw3t")
                nc.gpsimd.partition_broadcast(ptw3[:], ptr3[0][:])
                nc.gpsimd.partition_broadcast(ptw3t[:], ptr3[1][:])
                nc.vector.tensor_copy(ptw3[CAP:128, :], ptw3t[CAP:128, :])

                k2P = []
                k2D = []
                for q in range(2):
                    P = big.tile([128, 3 * N], f32, tag=f"k2P{q}", name=f"k2P{q}")
                    colap = bass.AP(tensor=ptcolf.tensor,
                                    offset=ptcolf.offset + q * 3,
                                    ap=[list(ptcolf.ap[0]), [1, 3], [0, N]])
                    rowap = bass.AP(tensor=ptrow2b[q].tensor,
                                    offset=ptrow2b[q].offset,
                                    ap=[list(ptrow2b[q].ap[0]), [0, 3], [1, N]])
                    nc.vector.tensor_tensor(P[:], colap, rowap, OP.mult)
                    Dt = big.tile([128, 3 * N], f32, tag=f"k2D{q}", name=f"k2D{q}")
                    nc.vector.tensor_tensor(Dt[:], E_ttf[:], P[:], OP.mult)
                    k2P.append(P)
                    k2D.append(Dt)

                k3D = big.tile([128, N], f32, tag="k3D", name="k3D")
                nc.vector.tensor_tensor(k3D[:], E_pt[:], ptw3[:], OP.mult)

            if STAGE >= 30:
                # ---------------- partial sums + collective ----------------
                part = sm.tile([C, 36], f32, tag="part", name="part")

                def small_chain(lhs_tile, rhs_ap, n_free, rhs2_tiles, acc_ps,
                                first, last):
                    ap_ = pA.tile([C, 768], f32, tag="A", name="ap_")
                    done = 0
                    while done < n_free:
                        nchunk = min(512, n_free - done)
                        nc.tensor.matmul(ap_[:, done:done + nchunk], lhs_tile[:],
                                         rhs_ap[:, done:done + nchunk],
                                         start=True, stop=True)
                        done += nchunk
                    asb = scr.tile([C, 768], f32, tag="Asb", name="asb")
                    nc.scalar.copy(asb[:, :n_free], ap_[:, :n_free])
                    nblk = n_free // 128
                    for m in range(nblk):
                        tp_ = pT.tile([128, C], f32, tag="tiny", name="tpA")
                        nc.tensor.transpose(tp_[:], asb[:, m * 128:(m + 1) * 128],
                                            eye12[:])
                        atsb = scr.tile([128, C], f32, tag="ATsb", name="atsb")
                        nc.vector.tensor_copy(atsb[:], tp_[:])
                        nc.tensor.matmul(acc_ps[:], atsb[:], rhs2_tiles[m][:],
                                         start=(first and m == 0),
                                         stop=(last and m == nblk - 1))

                s1ps = pS.tile([C, C], f32, tag="S", name="s1ps")
                small_chain(wown, E_own, R, wrb, s1ps, True, True)
                nc.vector.tensor_copy(part[:, 0:12], s1ps[:])

                stps = pS.tile([C, C], f32, tag="S", name="stps")
                for blk in range(3):
                    small_chain(ptb[blk], E_ttf[:, blk * N:(blk + 1) * N], N, ptb,
                                stps, blk == 0, blk == 2)
                nc.vector.tensor_scalar_mul(part[:, 12:24], stps[:], 1.0 / NCORES)

                ssps = pS.tile([C, C], f32, tag="S", name="ssps")
                small_chain(wown, E_pt, N, ptb, ssps, True, True)
                nc.vector.tensor_copy(part[:, 24:36], ssps[:])

                d_ccin = dpool.tile([C, 36], f32, tag="d_ccin", name="d_ccin")
                d_ccout = dpool.tile([C, 36], f32, tag="d_ccout", name="d_ccout")
                dma(out=d_ccin[:], in_=part[:])
                nc.gpsimd.collective_compute(
                    "AllReduce", mybir.AluOpType.add,
                    replica_groups=[list(range(NCORES))],
                    ins=[d_ccin.opt()], outs=[d_ccout.opt()])
                sred = sm.tile([C, 36], f32, tag="sred", name="sred")
                dma(out=sred[:], in_=d_ccout[:])
                dma(out=o_sred[:], in_=sred[:])

            if STAGE >= 40:
                # ---------------- gammas ----------------
                S1 = sred[:, 0:12]
                sttM = sred[:, 12:24]
                sstM = sred[:, 24:36]

                def diag_col(mat, nm):
                    s_ = scr.tile([C, C], f32, tag="diagscr", name="dsc")
                    col = sm.tile([C, 1], f32, tag=nm, name=nm)
                    nc.vector.tensor_tensor(s_[:], mat, eye12[:], OP.mult)
                    nc.vector.reduce_sum(out=col[:], in_=s_[:],
                                         axis=mybir.AxisListType.X)
                    return col

                ssscol = diag_col(S1, "ssscol")
                sttcol = diag_col(sttM, "sttcol")
                sstdcol = diag_col(sstM, "sstdcol")

                gin = sm.tile([C, 1], f32, tag="gin", name="gin")
                nc.vector.scalar_tensor_tensor(out=gin[:], in0=sstdcol[:], scalar=2.0,
                                               in1=sttcol[:], op0=OP.mult, op1=OP.add)
                nc.vector.tensor_tensor(gin[:], gin[:], ssscol[:], OP.add)
                nc.vector.tensor_tensor(gin[:], gin[:], rdenin[:], OP.mult)

                ssst = pT.tile([1, C], f32, tag="tiny", name="ssst")
                nc.tensor.transpose(ssst[:], ssscol[:], eye12[:])
                ssstsb = sm.tile([1, C], f32, tag="ssstsb", name="ssstsb")
                nc.vector.tensor_copy(ssstsb[:], ssst[:])
                sssrowb = sm.tile([C, C], f32, tag="sssrowb", name="sssrowb")
                nc.gpsimd.partition_broadcast(sssrowb[:], ssstsb[:])
                g2 = sm.tile([C, C], f32, tag="g2", name="g2")
                nc.vector.tensor_scalar(g2[:], S1, 2.0, None, OP.mult)
                nc.vector.tensor_tensor(g2[:], g2[:], sssrowb[:], OP.add)
                nc.vector.tensor_scalar(g2[:], g2[:], ssscol[:], None, OP.add)
                nc.vector.tensor_tensor(g2[:], g2[:], rden2[:], OP.mult)
                dma(out=o_g2[:], in_=g2[:])

                # IBG [12, 65] = -1/bw : cols 0-59 from g2 (k-major), 60-64 from gin
                ibg0 = sm.tile([C, 65], f32, tag="ibg0", name="ibg0")
                g2ap = g2[:]
                g2exp = bass.AP(tensor=g2ap.tensor, offset=g2ap.offset,
                                ap=[list(g2ap.ap[0]), [0, 5], [1, 12]])
                nc.vector.tensor_tensor(ibg0[:, 0:60], g2exp, pw60[:], OP.mult)
                ginap = gin[:]
                ginexp = bass.AP(tensor=ginap.tensor, offset=ginap.offset,
                                 ap=[list(ginap.ap[0]), [0, 5]])
                nc.vector.tensor_tensor(ibg0[:, 60:65], ginexp, pw5[:], OP.mult)
                nc.vector.tensor_scalar(ibg0[:], ibg0[:], -1e-5, None, OP.min)
                ibg = sm.tile([C, 65], f32, tag="ibg", name="ibg")
                nc.vector.reciprocal(ibg[:], ibg0[:])
                dma(out=o_ibg[:], in_=ibg[:])

                selsb = []
                for h in range(2):
                    ps_ = pT.tile([1, 65], f32, tag="tiny", name="psel")
                    nc.tensor.matmul(ps_[:], oh2[:, h:h + 1], ibg[:],
                                     start=True, stop=True)
                    s_ = sm.tile([1, 65], f32, tag=f"sel{h}", name=f"sel{h}")
                    nc.vector.tensor_copy(s_[:], ps_[:])
                    selsb.append(s_)

                sclT1 = sm.tile([128, 1], f32, tag="sclT1", name="sclT1")
                sclT3 = sm.tile([128, 1], f32, tag="sclT3", name="sclT3")
                nc.vector.memset(sclT1[:], 0.0)
                nc.vector.memset(sclT3[:], 0.0)
                negk1 = sm.tile([128, 5], f32, tag="negk1", name="negk1")
                for h in range(2):
                    tp_ = pT.tile([65, 1], f32, tag="tiny", name="tsel")
                    nc.tensor.transpose(tp_[:], selsb[h][:], eye128[0:1, 0:1])
                    tpsb = scr.tile([65, 1], f32, tag="tselsb", name="tpsb")
                    nc.vector.tensor_copy(tpsb[:], tp_[:])
                    nc.vector.tensor_copy(sclT1[h * 64:h * 64 + 60, :], tpsb[0:60, :])
                    pp_ = pT.tile([1, 65], f32, tag="tiny", name="pp_")
                    nc.tensor.matmul(pp_[:], tpsb[:], perm65[:], start=True, stop=True)
                    ppsb = scr.tile([1, 65], f32, tag="ppermsb", name="ppsb")
                    nc.vector.tensor_copy(ppsb[:], pp_[:])
                    tp2 = pT.tile([65, 1], f32, tag="tiny", name="tp2")
                    nc.tensor.transpose(tp2[:], ppsb[:], eye128[0:1, 0:1])
                    tp2sb = scr.tile([65, 1], f32, tag="tsel2sb", name="tp2sb")
                    nc.vector.tensor_copy(tp2sb[:], tp2[:])
                    nc.vector.tensor_copy(sclT3[h * 64:h * 64 + 60, :], tp2sb[0:60, :])
                    nkt = sm.tile([128, 5], f32, tag=f"negk1t{h}",
                                  name=f"nkt{h}")
                    nc.gpsimd.partition_broadcast(nkt[:], selsb[h][0:1, 60:65])
                    if h == 0:
                        nc.vector.tensor_copy(negk1[0:CAP, :], nkt[0:CAP, :])
                    else:
                        nc.vector.tensor_copy(negk1[CAP:128, :], nkt[CAP:128, :])

                negb = []
                for q in range(2):
                    k2sc = pT.tile([1, 5], f32, tag="tiny", name="k2sc")
                    nc.tensor.matmul(k2sc[:], k2sel[:, q:q + 1], ibg[:, 60:65],
                                     start=True, stop=True)
                    k2scsb = sm.tile([1, 5], f32, tag=f"k2scsb{q}", name=f"k2scsb{q}")
                    nc.vector.tensor_copy(k2scsb[:], k2sc[:])
                    nb = sm.tile([128, 5], f32, tag=f"negb{q}", name=f"negb{q}")
                    nc.gpsimd.partition_broadcast(nb[:], k2scsb[:])
                    negb.append(nb)

            if STAGE >= 50:
                # ---------------- ACC + exp passes ----------------
                acc = big.tile([128, NCOL], f32, tag="acc", name="acc")
                nc.vector.memset(acc[:], 0.0)

                nc.scalar.activation(t1src[:], t1src[:], AF.Exp, scale=sclT1[:],
                                     accum_out=acc[:, 0:1])
                nc.scalar.activation(t3src[:], t3src[:], AF.Exp, scale=sclT3[:],
                                     accum_out=acc[:, 1:2])

                for k in range(KN):
                    sk = scr.tile([128, CAP], f32, tag="k1scr", name="sk1")
                    nc.scalar.activation(sk[:], E_diag[:], AF.Exp,
                                         scale=negk1[:, k:k + 1],
                                         accum_out=acc[:, 2 + k:3 + k])

                for k in range(KN):
                    ek = scr.tile([128, N], f32, tag="k3e", name="ek3")
                    nc.scalar.activation(ek[:], k3D[:], AF.Exp,
                                         scale=negk1[:, k:k + 1])
                    sk = scr.tile([128, N], f32, tag="k3scr", name="sk3")
                    nc.vector.scalar_tensor_tensor(
                        out=sk[:], in0=ek[:], scalar=1.0, in1=ptw3[:],
                        op0=OP.mult, op1=OP.mult,
                        accum_out=acc[:, 7 + k:8 + k])

                for q in range(2):
                    e0 = scr.tile([128, 3 * N], f32, tag="k2acc", name="e0")
                    nc.scalar.activation(e0[:], k2D[q][:], AF.Exp,
                                         scale=negb[q][:, 0:1])
                    for k in range(1, KN):
                        ek = scr.tile([128, 3 * N], f32, tag="k2e", name="ek2")
                        nc.scalar.activation(ek[:], k2D[q][:], AF.Exp,
                                             scale=negb[q][:, k:k + 1])
                        nc.vector.tensor_tensor(e0[:], e0[:], ek[:], OP.add)
                    sk = scr.tile([128, 3 * N], f32, tag="k2scr", name="sk2")
                    nc.vector.scalar_tensor_tensor(
                        out=sk[:], in0=e0[:], scalar=1.0, in1=k2P[q][:],
                        op0=OP.mult, op1=OP.mult,
                        accum_out=acc[:, 12 + q:13 + q])

                dma(out=o_acc[:], in_=acc[:])

                # ---------------- final weighted reduce ----------------
                v = big.tile([128, NCOL], f32, tag="v", name="v")
                nc.vector.tensor_tensor(v[:], acc[:], wm[:], OP.mult)
                m1 = pT.tile([NCOL, 1], f32, tag="tiny", name="m1")
                nc.tensor.matmul(m1[:], v[:], ones[:], start=True, stop=True)
                m1sb = sm.tile([NCOL, 1], f32, tag="m1sb", name="m1sb")
                nc.vector.tensor_copy(m1sb[:], m1[:])
                m2 = pT.tile([1, 2], f32, tag="tiny", name="m2")
                nc.tensor.matmul(m2[:], m1sb[:], ssel[:], start=True, stop=True)
                res = sm.tile([1, 2], f32, tag="res", name="res")
                nc.vector.tensor_tensor(res[:], m2[:], offs[:], OP.add)
                dma(out=o_out[:], in_=res[:])
            if STAGE < 50:
                dma(out=o_out[:], in_=wm[0:1, 0:2])

    nc.compile()
    return nc


def get_program():
    import os
    key = ("nc", os.environ.get("CDD_STAGE", "99"))
    if key not in _COMPILED:
        _COMPILED[key] = _build_program()
    return _COMPILED[key]


# ----------------------------------------------------------------------------
# entry point
# ----------------------------------------------------------------------------

def _run(in_maps, trace=False):
    from concourse.bass_utils import run_bass_kernel_spmd
    nc = get_program()
    return run_bass_kernel_spmd(nc, in_maps, list(range(NCORES)), trace=trace)


def kernel(src_x, tgt_x, src_y, tgt_y):
    in_maps = _host_prep(src_x, tgt_x, src_y, tgt_y)
    if in_maps is None:
        return _numpy_fallback(src_x, tgt_x, src_y, tgt_y)
    br = _run(in_maps)
    total = np.zeros(2, np.float64)
    for res in br.results:
        total += res["out"].reshape(2).astype(np.float64)
    return total.astype(np.float32)



# revision 5
# speedup vs baseline: 1.2999x; 1.2999x over previous
"""CDD loss kernel for 8 Trainium2 NeuronCores (Bass/Tile, SPMD).

Math (validated vs reference in float32):
  ps is one-hot -> every (C,C,N,N) reference tensor collapses to per-class-
  block sums. Host sorts+pads src rows by class (CAP rows/class, pads are
  huge distinct sentinel vectors so exp(-dist/bw) underflows to exactly 0).
  The E_pp class-diagonal blocks have their diagonal zeroed on device, making
  each diagonal entry contribute exactly exp(0)=1 per bandwidth; the exact
  correction (5*CAP - 5*exp(-1e-5)*cs) is applied as a host-computed offset.
  g2 is symmetric -> T2 = T1^T, so inter = sum_{s!=t} 2*(T1-T3)/(C^2-C).

Distribution (SPMD, one program, per-core data):
  - every core computes E rows for its class pair (rotation of the padded
    src rows makes "own" rows/cols sit at fixed offsets), partial
    S1 = Wown^T E_pp W, sst = Wown^T E_pt pt, stt = pt^T E_tt pt / 8
  - one AllReduce of the packed [12,36] partials, issued as soon as the
    partials exist so it overlaps the gather/build work
  - inputs are packed host-side into 3 DRAM tensors (3 loads, not 38)
  - E_own is written to DRAM in class-block layout so the T3 flat-block
    gather reads contiguous 16KB segments (2 DMAs, not 24); DMAs are
    spread across the SP and Activation HWDGE queues
  - exp-heavy sums run as single ACT instructions over flattened broadcast
    tiles with per-partition scale and accum_out; k2 runs as 5+3 per-pass
    (class,bandwidth) units per core (balanced across cores via a
    host-permuted bandwidth table folded into the ibg build)
  - per-core weighted reduce with host weight matrix -> [intra, inter]
    partials, host sums the 8 partials.
"""

import math
import numpy as np

C = 12
KN = 5
MU = 2
N = 384
D = 256
CAP = 64
R = C * CAP            # 768 padded src rows
NCORES = 8
NCOL = 20              # ACC columns: T1, T3, k1*5, k3*5, k2q0*5, k2q1*3
DIAG5 = 5.0 * math.exp(-1e-5)
I2 = 2.0 / (C * C - C)

# misc pack column offsets ([128, MISCW] host tensor)
O_EYE128 = 0
O_DIAGM = 128
O_WM = 192
O_ONES = 212
O_WR = 213
O_PTB = 285
O_PTCOLF = 321
O_WOWN = 327
O_PTRP = 339          # row 0, 4 x 384: ptr2a, ptr2b, ptr3a, ptr3b
O_EYE12 = 1875
O_OH2 = 1887
O_K2SEL = 1889
O_PERM65 = 1891       # rows 0-64
O_PW60 = 1956
O_PW10 = 2016
O_RDEN2 = 2026
O_RDENIN = 2038
O_SSEL = 2039         # rows 0-19
O_OFFS = 2041         # row 0
MISCW = 2043

_COMPILED = {}


# ----------------------------------------------------------------------------
# host-side prep
# ----------------------------------------------------------------------------

def _host_prep(src_x, tgt_x, src_y, tgt_y):
    src_x = np.ascontiguousarray(np.asarray(src_x, dtype=np.float32))
    tgt_x = np.ascontiguousarray(np.asarray(tgt_x, dtype=np.float32))
    src_y = np.asarray(src_y).astype(np.int64)
    pt = np.ascontiguousarray(np.asarray(tgt_y, dtype=np.float32))

    counts = np.bincount(src_y, minlength=C)
    if counts.max() > CAP:
        return None  # caller falls back to numpy path

    perm = np.argsort(src_y, kind="stable")
    sx_pad = np.zeros((R, D), np.float32)
    W = np.zeros((R, C), np.float32)
    # pad sentinels: huge random-sign vectors. Pad-pad dot products are then
    # tiny relative to the norms (no catastrophic cancellation in d2), every
    # pad-involved distance is >= ~3e5 and exp(-dist/bw) underflows to 0.
    rng = np.random.default_rng(987654321)
    sgn = (rng.integers(0, 2, size=(R, D)).astype(np.float32) * 2.0 - 1.0)
    off = 0
    padidx = 0
    for c in range(C):
        idx = perm[off:off + counts[c]]
        sx_pad[c * CAP:c * CAP + counts[c]] = src_x[idx]
        W[c * CAP:c * CAP + counts[c], c] = 1.0
        for p in range(CAP - counts[c]):
            sx_pad[c * CAP + counts[c] + p, :] = 2.0e4 * sgn[padidx]
            padidx += 1
        off += counts[c]

    cs = counts.astype(np.float64)
    ct = pt.sum(0).astype(np.float64)
    pss = cs * cs
    ptt = ct * ct

    rden2 = (1.0 / (pss[:, None] + pss[None, :]
                    + 2.0 * cs[:, None] * cs[None, :])).astype(np.float32)
    rdenin = (1.0 / (pss + ptt + 2.0 * cs * ct)).astype(np.float32).reshape(C, 1)

    pw5 = np.array([-(float(MU) ** (k - KN // 2)) for k in range(KN)],
                   np.float32)
    pw60 = np.zeros((C, 60), np.float32)
    for k in range(KN):
        pw60[:, k * 12:(k + 1) * 12] = pw5[k]

    tx_pack = np.ascontiguousarray(
        tgt_x.reshape(3, 128, D).transpose(1, 0, 2).reshape(128, 3 * D))

    in_maps = []
    for r in range(NCORES):
        g = r % 6
        a, b = 2 * g, 2 * g + 1
        pp_active = r < 6
        roll = 2 * g * CAP

        sxr = np.roll(sx_pad, -roll, axis=0)
        sxf_pack = np.ascontiguousarray(
            sxr.reshape(6, 128, D).transpose(1, 0, 2).reshape(128, 6 * D))
        wr = np.roll(W, -roll, axis=0)
        wown = wr[0:128].copy() if pp_active else np.zeros((128, C), np.float32)

        oh2 = np.zeros((C, 2), np.float32)
        oh2[a, 0] = 1.0
        oh2[b, 1] = 1.0

        # k2 split: q0 = class r with all 5 bandwidths; q1 = class 8+(r%4)
        # with bandwidths {0,1,2} on cores 0-3 and {3,4,dup} on cores 4-7.
        c_q0 = r
        c_q1 = 8 + (r % 4)
        kq1 = [0, 1, 2] if r < 4 else [3, 4]
        k2sel = np.zeros((C, 2), np.float32)
        k2sel[c_q0, 0] = 1.0
        k2sel[c_q1, 1] = 1.0
        pw10 = np.zeros((C, 10), np.float32)
        pw10[:, 0:5] = pw5[None, :]
        for j in range(5):
            pw10[:, 5 + j] = pw5[kq1[j]] if j < len(kq1) else pw5[0]

        ptrow2 = np.zeros((2, N), np.float32)
        ptcolf = np.zeros((128, 6), np.float32)
        for q, c in enumerate((c_q0, c_q1)):
            ptrow2[q] = pt[:, c]
            for blk in range(3):
                ptcolf[:, q * 3 + blk] = pt[blk * 128:(blk + 1) * 128, c]

        # perm65: sclT3[j] = ibg[cls, perm(j)] via matmul(lhsT=perm65, rhs=selcol)
        # row layout j = k*12 + t; source col = k*12 + rot(t), rot(t)=(2g+t)%12
        perm65 = np.zeros((65, 65), np.float32)
        for k in range(KN):
            for t in range(12):
                perm65[k * 12 + ((2 * g + t) % 12), k * 12 + t] = 1.0
        for j in range(60, 65):
            perm65[j, j] = 1.0

        wm = np.zeros((128, NCOL), np.float32)
        if pp_active:
            for h, cls in ((0, a), (1, b)):
                for k in range(KN):
                    for t in range(12):
                        if t != cls:
                            wm[h * 64 + k * 12 + t, 0] = I2 / pss[cls]
                        rt_ = (2 * g + t) % 12
                        if rt_ != cls:
                            wm[h * 64 + k * 12 + t, 1] = \
                                -I2 / (cs[cls] * cs[rt_])
                for k in range(KN):
                    wm[h * CAP:(h + 1) * CAP, 2 + k] = 1.0 / (C * pss[cls])
                    wm[h * CAP:(h + 1) * CAP, 7 + k] = \
                        -2.0 / (C * cs[cls] * ct[cls])
        wm[:, 12:17] = 1.0 / (C * ptt[c_q0])
        for j in range(len(kq1)):
            wm[:, 17 + j] = 1.0 / (C * ptt[c_q1])

        ssel = np.zeros((NCOL, 2), np.float32)
        ssel[2:NCOL, 0] = 1.0   # intra cols: k1, k3, k2
        ssel[0:2, 1] = 1.0      # inter cols: T1, T3

        offs = np.zeros((1, 2), np.float32)
        if r == 0:
            corr = 5.0 * CAP - DIAG5 * cs
            offs[0, 0] = -(corr / pss / C).sum()
            offs[0, 1] = -((C - 1) * corr * I2 / pss).sum()

        misc = np.zeros((128, MISCW), np.float32)
        misc[:, O_EYE128:O_EYE128 + 128] = np.eye(128, dtype=np.float32)
        misc[0:CAP, O_DIAGM:O_DIAGM + CAP] = 1.0 - np.eye(CAP, dtype=np.float32)
        misc[CAP:128, O_DIAGM:O_DIAGM + CAP] = 1.0 - np.eye(CAP, dtype=np.float32)
        misc[:, O_WM:O_WM + NCOL] = wm
        misc[:, O_ONES] = 1.0
        for m in range(6):
            misc[:, O_WR + m * 12:O_WR + (m + 1) * 12] = \
                wr[m * 128:(m + 1) * 128]
        for m in range(3):
            misc[:, O_PTB + m * 12:O_PTB + (m + 1) * 12] = \
                pt[m * 128:(m + 1) * 128]
        misc[:, O_PTCOLF:O_PTCOLF + 6] = ptcolf
        misc[:, O_WOWN:O_WOWN + 12] = wown
        misc[0, O_PTRP:O_PTRP + N] = ptrow2[0]
        misc[0, O_PTRP + N:O_PTRP + 2 * N] = ptrow2[1]
        misc[0, O_PTRP + 2 * N:O_PTRP + 3 * N] = pt[:, a]
        misc[0, O_PTRP + 3 * N:O_PTRP + 4 * N] = pt[:, b]
        misc[0:12, O_EYE12:O_EYE12 + 12] = np.eye(C, dtype=np.float32)
        misc[0:12, O_OH2:O_OH2 + 2] = oh2
        misc[0:12, O_K2SEL:O_K2SEL + 2] = k2sel
        misc[0:65, O_PERM65:O_PERM65 + 65] = perm65
        misc[0:12, O_PW60:O_PW60 + 60] = pw60
        misc[0:12, O_PW10:O_PW10 + 10] = pw10
        misc[0:12, O_RDEN2:O_RDEN2 + 12] = rden2
        misc[0:12, O_RDENIN:O_RDENIN + 1] = rdenin
        misc[0:NCOL, O_SSEL:O_SSEL + 2] = ssel
        misc[0, O_OFFS:O_OFFS + 2] = offs[0]

        in_maps.append({
            "sxfp": sxf_pack,
            "txp": tx_pack,
            "misc": np.ascontiguousarray(misc),
        })
    return in_maps


def _numpy_fallback(src_x, tgt_x, src_y, tgt_y):
    f = np.float32
    src_x = np.asarray(src_x, f)
    tgt_x = np.asarray(tgt_x, f)
    src_y = np.asarray(src_y).astype(np.int64)
    pt = np.asarray(tgt_y, f)
    ps = np.eye(C, dtype=f)[src_y]

    def cdist(a, bb):
        d2 = (a * a).sum(1)[:, None] + (bb * bb).sum(1)[None, :] - 2.0 * (a @ bb.T)
        return np.sqrt(np.maximum(d2, 0.0))

    def kern(dist, g):
        acc = 0.0
        for i in range(KN):
            bw = np.maximum(np.asarray(g) * (MU ** (i - KN // 2)), 1e-5)
            acc = acc + np.exp(-np.clip(dist / bw, 1e-5, 1e5))
        return acc

    E_ss = cdist(src_x, src_x); E_tt = cdist(tgt_x, tgt_x); E_st = cdist(src_x, tgt_x)
    sss = np.einsum('ic,ij,jc->c', ps, E_ss, ps)
    stt = np.einsum('ic,ij,jc->c', pt, E_tt, pt)
    sst = np.einsum('is,ij,jt->st', ps, E_st, pt)
    cs = ps.sum(0); ct = pt.sum(0)
    pss = cs * cs; ptt = ct * ct; pstd = cs * ct
    g_in = (sss + stt + 2 * np.diagonal(sst)) / (pss + ptt + 2 * pstd)
    Pss = ps.T[:, :, None] * ps.T[:, None, :]
    Ptt = pt.T[:, :, None] * pt.T[:, None, :]
    Pst = ps.T[:, :, None] * pt.T[:, None, :]
    k1 = (kern(E_ss[None] * Pss, g_in[:, None, None]) * Pss).sum((-2, -1)) / pss
    k2 = (kern(E_tt[None] * Ptt, g_in[:, None, None]) * Ptt).sum((-2, -1)) / ptt
    k3 = (kern(E_st[None] * Pst, g_in[:, None, None]) * Pst).sum((-2, -1)) / pstd
    intra = (k1 + k2 - 2 * k3).sum() / C
    sst_s = np.einsum('is,ij,jt->st', ps, E_ss, ps)
    g2 = (sss[:, None] + sss[None, :] + 2 * sst_s) / (
        pss[:, None] + pss[None, :] + 2 * cs[:, None] * cs[None, :])
    T1 = np.zeros((C, C), f); T3 = np.zeros((C, C), f)
    for s in range(C):
        ms = ps[:, s].astype(bool)
        for t in range(C):
            mt = ps[:, t].astype(bool)
            T1[s, t] = kern(E_ss[np.ix_(ms, ms)], g2[s, t]).sum() / pss[s]
            T3[s, t] = kern(E_ss[np.ix_(ms, mt)], g2[s, t]).sum() / (cs[s] * cs[t])
    inter = ((2 * T1 - 2 * T3) * (1 - np.eye(C))).sum() / (C * C - C)
    return np.array([intra, inter], np.float32)


# ----------------------------------------------------------------------------
# device program
# ----------------------------------------------------------------------------

def _build_program():
    import concourse.bass as bass
    import concourse.tile as tile
    from concourse import bacc, mybir

    f32 = mybir.dt.float32
    AF = mybir.ActivationFunctionType
    OP = mybir.AluOpType

    nc = bacc.Bacc("TRN2", target_bir_lowering=False, debug=False,
                   num_devices=NCORES)

    i_sxfp = nc.dram_tensor("sxfp", [128, 6 * D], f32, kind="ExternalInput").ap()
    i_txp = nc.dram_tensor("txp", [128, 3 * D], f32, kind="ExternalInput").ap()
    i_misc = nc.dram_tensor("misc", [128, MISCW], f32, kind="ExternalInput").ap()

    o_out = nc.dram_tensor("out", [1, 2], f32, kind="ExternalOutput").ap()

    with tile.TileContext(nc) as tc:
        with (
            tc.tile_pool(name="io", bufs=1) as io,
            tc.tile_pool(name="big", bufs=1) as big,
            tc.tile_pool(name="scr", bufs=2) as scr,
            tc.tile_pool(name="sm", bufs=1) as sm,
            tc.tile_pool(name="pG", bufs=2, space="PSUM") as pG,
            tc.tile_pool(name="pA", bufs=1, space="PSUM") as pA,
            tc.tile_pool(name="pT", bufs=2, space="PSUM") as pT,
            tc.tile_pool(name="pS", bufs=1, space="PSUM") as pS,
            tc.tile_pool(name="dram", bufs=1, space="DRAM") as dpool,
        ):
            dma_sp = nc.sync.dma_start
            dma_act = nc.scalar.dma_start

            # ---------------- input loads: 3 big DMAs ----------------
            sxfp = io.tile([128, 6 * D], f32, tag="sxfp", name="sxfp")
            dma_sp(out=sxfp[:], in_=i_sxfp[:])
            txp = io.tile([128, 3 * D], f32, tag="txp", name="txp")
            dma_act(out=txp[:], in_=i_txp[:])
            misc = io.tile([128, MISCW], f32, tag="misc", name="misc")
            dma_sp(out=misc[:], in_=i_misc[:])

            eye128 = misc[:, O_EYE128:O_EYE128 + 128]
            diagm = misc[:, O_DIAGM:O_DIAGM + CAP]
            wm = misc[:, O_WM:O_WM + NCOL]
            ones = misc[:, O_ONES:O_ONES + 1]
            wrb = [misc[:, O_WR + m * 12:O_WR + (m + 1) * 12] for m in range(6)]
            ptb = [misc[:, O_PTB + m * 12:O_PTB + (m + 1) * 12] for m in range(3)]
            pcf = misc[:, O_PTCOLF:O_PTCOLF + 6]
            wown = misc[:, O_WOWN:O_WOWN + 12]
            ptr2 = [misc[0:1, O_PTRP + q * N:O_PTRP + (q + 1) * N]
                    for q in range(2)]
            ptr3 = [misc[0:1, O_PTRP + (q + 2) * N:O_PTRP + (q + 3) * N]
                    for q in range(2)]
            eye12 = misc[0:12, O_EYE12:O_EYE12 + 12]
            oh2 = misc[0:12, O_OH2:O_OH2 + 2]
            k2sel = misc[0:12, O_K2SEL:O_K2SEL + 2]
            perm65 = misc[0:65, O_PERM65:O_PERM65 + 65]
            pw60 = misc[0:12, O_PW60:O_PW60 + 60]
            pw10 = misc[0:12, O_PW10:O_PW10 + 10]
            rden2 = misc[0:12, O_RDEN2:O_RDEN2 + 12]
            rdenin = misc[0:12, O_RDENIN:O_RDENIN + 1]
            ssel = misc[0:NCOL, O_SSEL:O_SSEL + 2]
            offs = misc[0:1, O_OFFS:O_OFFS + 2]

            # ---------------- transposes: sxfT, txT ----------------
            sxfT = [big.tile([128, R], f32, tag=f"sxfT{k}", name=f"sxfT{k}")
                    for k in range(2)]
            txT = [big.tile([128, N], f32, tag=f"txT{k}", name=f"txT{k}")
                   for k in range(2)]
            for m in range(6):
                for k in range(2):
                    tp_ = pT.tile([128, 128], f32, tag="tiny", name="tp")
                    nc.tensor.transpose(
                        tp_[:], sxfp[:, m * 256 + k * 128:m * 256 + (k + 1) * 128],
                        eye128)
                    nc.vector.tensor_copy(sxfT[k][:, m * 128:(m + 1) * 128], tp_[:])
            for m in range(3):
                for k in range(2):
                    tp_ = pT.tile([128, 128], f32, tag="tiny", name="tp")
                    nc.tensor.transpose(
                        tp_[:], txp[:, m * 256 + k * 128:m * 256 + (k + 1) * 128],
                        eye128)
                    nc.vector.tensor_copy(txT[k][:, m * 128:(m + 1) * 128], tp_[:])

            # ---------------- row norms ----------------
            rscol = [sm.tile([128, 1], f32, tag=f"rs{m}", name=f"rs{m}")
                     for m in range(6)]
            rtcol = [sm.tile([128, 1], f32, tag=f"rt{m}", name=f"rt{m}")
                     for m in range(3)]
            for m in range(6):
                nsc = scr.tile([128, D], f32, tag="normscr", name="nsc")
                nc.scalar.activation(nsc[:], sxfp[:, m * 256:(m + 1) * 256],
                                     AF.Square, accum_out=rscol[m][:])
            for m in range(3):
                nsc = scr.tile([128, D], f32, tag="normscr", name="nsc")
                nc.scalar.activation(nsc[:], txp[:, m * 256:(m + 1) * 256],
                                     AF.Square, accum_out=rtcol[m][:])

            rsrow = sm.tile([1, R], f32, tag="rsrow", name="rsrow")
            rtrow = sm.tile([1, N], f32, tag="rtrow", name="rtrow")
            for m in range(6):
                tp_ = pT.tile([1, 128], f32, tag="tiny", name="tpr")
                nc.tensor.transpose(tp_[:], rscol[m][:], eye128)
                nc.vector.tensor_copy(rsrow[:, m * 128:(m + 1) * 128], tp_[:])
            for m in range(3):
                tp_ = pT.tile([1, 128], f32, tag="tiny", name="tpr")
                nc.tensor.transpose(tp_[:], rtcol[m][:], eye128)
                nc.vector.tensor_copy(rtrow[:, m * 128:(m + 1) * 128], tp_[:])

            rsrowb = big.tile([128, R], f32, tag="rsrowb", name="rsrowb")
            rtrowb = big.tile([128, N], f32, tag="rtrowb", name="rtrowb")
            nc.gpsimd.partition_broadcast(rsrowb[:], rsrow[:])
            nc.gpsimd.partition_broadcast(rtrowb[:], rtrow[:])

            # early gpsimd broadcasts for the k2/k3 builds (must precede the
            # collective in the gpsimd queue so they don't stall behind it)
            ptrow2b = [big.tile([128, N], f32, tag=f"ptrow2b{q}",
                                name=f"ptrow2b{q}") for q in range(2)]
            nc.gpsimd.partition_broadcast(ptrow2b[0][:], ptr2[0])
            nc.gpsimd.partition_broadcast(ptrow2b[1][:], ptr2[1])
            ptw3 = big.tile([128, N], f32, tag="ptw3", name="ptw3")
            ptw3t = big.tile([128, N], f32, tag="ptw3t", name="ptw3t")
            nc.gpsimd.partition_broadcast(ptw3[:], ptr3[0])
            nc.gpsimd.partition_broadcast(ptw3t[:], ptr3[1])
            nc.vector.tensor_copy(ptw3[CAP:128, :], ptw3t[CAP:128, :])

            # ---------------- E matrices ----------------
            def emit_E(dst, lhsT_tiles, lhs_lo, rhs_tiles, n_cols, rcol, rowb):
                done = 0
                while done < n_cols:
                    nchunk = min(512, n_cols - done)
                    gp = pG.tile([128, 512], f32, tag="G", name="gp")
                    for k in range(2):
                        nc.tensor.matmul(
                            gp[:, :nchunk],
                            lhsT_tiles[k][:, lhs_lo:lhs_lo + 128],
                            rhs_tiles[k][:, done:done + nchunk],
                            start=(k == 0), stop=(k == 1))
                    t1_ = scr.tile([128, 512], f32, tag="d2scr", name="d2s")
                    nc.vector.scalar_tensor_tensor(
                        out=t1_[:, :nchunk], in0=gp[:, :nchunk], scalar=-2.0,
                        in1=rowb[:, done:done + nchunk],
                        op0=OP.mult, op1=OP.add)
                    nc.vector.tensor_scalar(
                        t1_[:, :nchunk], t1_[:, :nchunk],
                        rcol[:], 0.0, OP.add, OP.max)
                    nc.scalar.activation(dst[:, done:done + nchunk],
                                         t1_[:, :nchunk], AF.Sqrt)
                    done += nchunk

            E_own = big.tile([128, R], f32, tag="E_own", name="E_own")
            emit_E(E_own, sxfT, 0, sxfT, R, rscol[0], rsrowb)

            E_ttf = big.tile([128, 3 * N], f32, tag="E_ttf", name="E_ttf")
            for blk in range(3):
                emit_E(E_ttf[:, blk * N:(blk + 1) * N], txT, blk * 128, txT, N,
                       rtcol[blk], rtrowb)

            E_pt = big.tile([128, N], f32, tag="E_pt", name="E_pt")
            emit_E(E_pt, sxfT, 0, txT, N, rscol[0], rtrowb)

            # ---------------- partial sums + collective (EARLY) ----------------
            part = sm.tile([C, 36], f32, tag="part", name="part")

            def small_chain(lhs_ap, rhs_ap, n_free, rhs2_aps, acc_ps,
                            first, last):
                ap_ = pA.tile([C, 768], f32, tag="A", name="ap_")
                done = 0
                while done < n_free:
                    nchunk = min(512, n_free - done)
                    nc.tensor.matmul(ap_[:, done:done + nchunk], lhs_ap,
                                     rhs_ap[:, done:done + nchunk],
                                     start=True, stop=True)
                    done += nchunk
                asb = scr.tile([C, 768], f32, tag="Asb", name="asb")
                nc.scalar.copy(asb[:, :n_free], ap_[:, :n_free])
                nblk = n_free // 128
                for m in range(nblk):
                    tp_ = pT.tile([128, C], f32, tag="tiny", name="tpA")
                    nc.tensor.transpose(tp_[:], asb[:, m * 128:(m + 1) * 128],
                                        eye12)
                    atsb = scr.tile([128, C], f32, tag="ATsb", name="atsb")
                    nc.vector.tensor_copy(atsb[:], tp_[:])
                    nc.tensor.matmul(acc_ps[:], atsb[:], rhs2_aps[m],
                                     start=(first and m == 0),
                                     stop=(last and m == nblk - 1))

            s1ps = pS.tile([C, C], f32, tag="S", name="s1ps")
            small_chain(wown, E_own, R, wrb, s1ps, True, True)
            nc.vector.tensor_copy(part[:, 0:12], s1ps[:])

            stps = pS.tile([C, C], f32, tag="S", name="stps")
            for blk in range(3):
                small_chain(ptb[blk], E_ttf[:, blk * N:(blk + 1) * N], N, ptb,
                            stps, blk == 0, blk == 2)
            nc.vector.tensor_scalar_mul(part[:, 12:24], stps[:], 1.0 / NCORES)

            ssps = pS.tile([C, C], f32, tag="S", name="ssps")
            small_chain(wown, E_pt, N, ptb, ssps, True, True)
            nc.vector.tensor_copy(part[:, 24:36], ssps[:])

            d_ccin = dpool.tile([C, 36], f32, tag="d_ccin", name="d_ccin")
            d_ccout = dpool.tile([C, 36], f32, tag="d_ccout", name="d_ccout")
            dma_sp(out=d_ccin[:], in_=part[:])
            nc.gpsimd.collective_compute(
                "AllReduce", mybir.AluOpType.add,
                replica_groups=[list(range(NCORES))],
                ins=[d_ccin.opt()], outs=[d_ccout.opt()])
            sred = sm.tile([C, 36], f32, tag="sred", name="sred")
            dma_sp(out=sred[:], in_=d_ccout[:])

            # ---------------- gather/build work (overlaps the collective) ----
            # diag-zeroed own-class diagonal blocks [128, 64]
            E_diag = big.tile([128, CAP], f32, tag="E_diag", name="E_diag")
            nc.vector.tensor_tensor(E_diag[0:CAP, :], E_own[0:CAP, 0:CAP],
                                    diagm[0:CAP, :], OP.mult)
            nc.vector.tensor_tensor(E_diag[CAP:128, :],
                                    E_own[CAP:128, CAP:128],
                                    diagm[CAP:128, :], OP.mult)

            # E_diag -> DRAM (row-major) for the T1 flat broadcast
            d_ed = dpool.tile([128, CAP], f32, tag="d_ed", name="d_ed")
            dma_sp(out=d_ed[:], in_=E_diag[:])
            # E_own -> DRAM in class-block layout [t][row][col] so each
            # (half, t) block is one contiguous 16KB segment
            d_eob = dpool.tile([C, 128 * CAP], f32, tag="d_eob", name="d_eob")
            dst_eob = bass.AP(tensor=d_eob.tensor, offset=0,
                              ap=[[CAP, 128], [128 * CAP, 12], [1, CAP]])
            dma_act(out=dst_eob, in_=E_own[:])

            t1src = big.tile([128, CAP * CAP], f32, tag="t1src", name="t1src")
            for h in range(2):
                ap_in = bass.AP(tensor=d_ed.tensor, offset=h * CAP * CAP,
                                ap=[[0, 64], [1, CAP * CAP]])
                dma_sp(out=t1src[h * 64:(h + 1) * 64, :], in_=ap_in)

            # T3 flat blocks: partition h*64 + k*12 + t <- block (h, t),
            # contiguous 4096-elem reads from the blocked layout
            t3src = big.tile([128, CAP * CAP], f32, tag="t3src", name="t3src")
            for h in range(2):
                ap_in = bass.AP(tensor=d_eob.tensor, offset=h * CAP * CAP,
                                ap=[[0, 5], [128 * CAP, 12], [1, CAP * CAP]])
                dma_act(out=t3src[h * 64:h * 64 + 60, :], in_=ap_in)
                ap_pad = bass.AP(tensor=d_eob.tensor, offset=h * CAP * CAP,
                                 ap=[[0, 4], [1, CAP * CAP]])
                dma_act(out=t3src[h * 64 + 60:h * 64 + 64, :], in_=ap_pad)

            # ---------------- k2 / k3 static builds ----------------
            k2P = []
            k2D = []
            for q in range(2):
                P = big.tile([128, 3 * N], f32, tag=f"k2P{q}", name=f"k2P{q}")
                colap = bass.AP(tensor=pcf.tensor,
                                offset=pcf.offset + q * 3,
                                ap=[list(pcf.ap[0]), [1, 3], [0, N]])
                rowap = bass.AP(tensor=ptrow2b[q].tensor,
                                offset=ptrow2b[q].offset,
                                ap=[list(ptrow2b[q].ap[0]), [0, 3], [1, N]])
                nc.vector.tensor_tensor(P[:], colap, rowap, OP.mult)
                Dt = big.tile([128, 3 * N], f32, tag=f"k2D{q}", name=f"k2D{q}")
                nc.vector.tensor_tensor(Dt[:], E_ttf[:], P[:], OP.mult)
                k2P.append(P)
                k2D.append(Dt)

            k3D = big.tile([128, N], f32, tag="k3D", name="k3D")
            nc.vector.tensor_tensor(k3D[:], E_pt[:], ptw3[:], OP.mult)

            acc = big.tile([128, NCOL], f32, tag="acc", name="acc")
            nc.vector.memset(acc[:], 0.0)
            sclT1 = sm.tile([128, 1], f32, tag="sclT1", name="sclT1")
            sclT3 = sm.tile([128, 1], f32, tag="sclT3", name="sclT3")
            nc.vector.memset(sclT1[:], 0.0)
            nc.vector.memset(sclT3[:], 0.0)

            # ---------------- gammas (post-collective) ----------------
            S1 = sred[:, 0:12]
            sttM = sred[:, 12:24]
            sstM = sred[:, 24:36]

            def diag_col(mat, nm):
                s_ = scr.tile([C, C], f32, tag="diagscr", name="dsc")
                col = sm.tile([C, 1], f32, tag=nm, name=nm)
                nc.vector.tensor_tensor(s_[:], mat, eye12, OP.mult)
                nc.vector.reduce_sum(out=col[:], in_=s_[:],
                                     axis=mybir.AxisListType.X)
                return col

            ssscol = diag_col(S1, "ssscol")
            sttcol = diag_col(sttM, "sttcol")
            sstdcol = diag_col(sstM, "sstdcol")

            gin = sm.tile([C, 1], f32, tag="gin", name="gin")
            nc.vector.scalar_tensor_tensor(out=gin[:], in0=sstdcol[:], scalar=2.0,
                                           in1=sttcol[:], op0=OP.mult, op1=OP.add)
            nc.vector.tensor_tensor(gin[:], gin[:], ssscol[:], OP.add)
            nc.vector.tensor_tensor(gin[:], gin[:], rdenin, OP.mult)

            ssst = pT.tile([1, C], f32, tag="tiny", name="ssst")
            nc.tensor.transpose(ssst[:], ssscol[:], eye12)
            ssstsb = sm.tile([1, C], f32, tag="ssstsb", name="ssstsb")
            nc.vector.tensor_copy(ssstsb[:], ssst[:])
            sssrowb = sm.tile([C, C], f32, tag="sssrowb", name="sssrowb")
            nc.gpsimd.partition_broadcast(sssrowb[:], ssstsb[:])
            g2 = sm.tile([C, C], f32, tag="g2", name="g2")
            nc.vector.tensor_scalar(g2[:], S1, 2.0, None, OP.mult)
            nc.vector.tensor_tensor(g2[:], g2[:], sssrowb[:], OP.add)
            nc.vector.tensor_scalar(g2[:], g2[:], ssscol[:], None, OP.add)
            nc.vector.tensor_tensor(g2[:], g2[:], rden2, OP.mult)

            # IBG [12, 70] = -1/bw : cols 0-59 from g2 (k-major),
            # 60-64 from gin (k-order), 65-69 from gin (host-permuted for q1)
            ibg0 = sm.tile([C, 70], f32, tag="ibg0", name="ibg0")
            g2ap = g2[:]
            g2exp = bass.AP(tensor=g2ap.tensor, offset=g2ap.offset,
                            ap=[list(g2ap.ap[0]), [0, 5], [1, 12]])
            nc.vector.tensor_tensor(ibg0[:, 0:60], g2exp, pw60, OP.mult)
            ginap = gin[:]
            ginexp = bass.AP(tensor=ginap.tensor, offset=ginap.offset,
                             ap=[list(ginap.ap[0]), [0, 10]])
            nc.vector.tensor_tensor(ibg0[:, 60:70], ginexp, pw10, OP.mult)
            nc.vector.tensor_scalar(ibg0[:], ibg0[:], -1e-5, None, OP.min)
            ibg = sm.tile([C, 70], f32, tag="ibg", name="ibg")
            nc.vector.reciprocal(ibg[:], ibg0[:])

            # per-half scale vectors
            negk1 = sm.tile([128, 5], f32, tag="negk1", name="negk1")
            for h in range(2):
                ps_ = pT.tile([1, 65], f32, tag="tiny", name="psel")
                nc.tensor.matmul(ps_[:], oh2[:, h:h + 1], ibg[:, 0:65],
                                 start=True, stop=True)
                selsb = sm.tile([1, 65], f32, tag=f"sel{h}", name=f"sel{h}")
                nc.vector.tensor_copy(selsb[:], ps_[:])
                tp_ = pT.tile([65, 1], f32, tag="tiny", name="tsel")
                nc.tensor.transpose(tp_[:], selsb[:], eye128[0:1, 0:1])
                tpsb = scr.tile([65, 1], f32, tag="tselsb", name="tpsb")
                nc.vector.tensor_copy(tpsb[:], tp_[:])
                nc.vector.tensor_copy(sclT1[h * 64:h * 64 + 60, :], tpsb[0:60, :])
                p3 = pT.tile([65, 1], f32, tag="tiny", name="p3")
                nc.tensor.matmul(p3[:], perm65, tpsb[:], start=True, stop=True)
                p3sb = scr.tile([65, 1], f32, tag="p3sb", name="p3sb")
                nc.vector.tensor_copy(p3sb[:], p3[:])
                nc.vector.tensor_copy(sclT3[h * 64:h * 64 + 60, :], p3sb[0:60, :])
                nkt = sm.tile([128, 5], f32, tag=f"negk1t{h}", name=f"nkt{h}")
                nc.gpsimd.partition_broadcast(nkt[:], selsb[0:1, 60:65])
                if h == 0:
                    nc.vector.tensor_copy(negk1[0:CAP, :], nkt[0:CAP, :])
                else:
                    nc.vector.tensor_copy(negk1[CAP:128, :], nkt[CAP:128, :])

            negb = []
            for q in range(2):
                k2sc = pT.tile([1, 5], f32, tag="tiny", name="k2sc")
                nc.tensor.matmul(k2sc[:], k2sel[:, q:q + 1],
                                 ibg[:, 60 + 5 * q:65 + 5 * q],
                                 start=True, stop=True)
                k2scsb = sm.tile([1, 5], f32, tag=f"k2scsb{q}", name=f"k2scsb{q}")
                nc.vector.tensor_copy(k2scsb[:], k2sc[:])
                nb = sm.tile([128, 5], f32, tag=f"negb{q}", name=f"negb{q}")
                nc.gpsimd.partition_broadcast(nb[:], k2scsb[:])
                negb.append(nb)

            # ---------------- exp passes ----------------
            nc.scalar.activation(t1src[:], t1src[:], AF.Exp, scale=sclT1[:],
                                 accum_out=acc[:, 0:1])
            nc.scalar.activation(t3src[:], t3src[:], AF.Exp, scale=sclT3[:],
                                 accum_out=acc[:, 1:2])

            for k in range(KN):
                sk = scr.tile([128, CAP], f32, tag="k1scr", name="sk1")
                nc.scalar.activation(sk[:], E_diag[:], AF.Exp,
                                     scale=negk1[:, k:k + 1],
                                     accum_out=acc[:, 2 + k:3 + k])

            for k in range(KN):
                ek = scr.tile([128, N], f32, tag="k3e", name="ek3")
                nc.scalar.activation(ek[:], k3D[:], AF.Exp,
                                     scale=negk1[:, k:k + 1])
                sk = scr.tile([128, N], f32, tag="k3scr", name="sk3")
                nc.vector.scalar_tensor_tensor(
                    out=sk[:], in0=ek[:], scalar=1.0, in1=ptw3[:],
                    op0=OP.mult, op1=OP.mult,
                    accum_out=acc[:, 7 + k:8 + k])

            for q in range(2):
                npass = 5 if q == 0 else 3
                for j in range(npass):
                    ek = scr.tile([128, 3 * N], f32, tag="k2e", name="ek2")
                    nc.scalar.activation(ek[:], k2D[q][:], AF.Exp,
                                         scale=negb[q][:, j:j + 1])
                    sk = scr.tile([128, 3 * N], f32, tag="k2scr", name="sk2")
                    col = 12 + 5 * q + j
                    nc.vector.scalar_tensor_tensor(
                        out=sk[:], in0=ek[:], scalar=1.0, in1=k2P[q][:],
                        op0=OP.mult, op1=OP.mult,
                        accum_out=acc[:, col:col + 1])

            # ---------------- final weighted reduce ----------------
            v = big.tile([128, NCOL], f32, tag="v", name="v")
            nc.vector.tensor_tensor(v[:], acc[:], wm, OP.mult)
            m1 = pT.tile([NCOL, 1], f32, tag="tiny", name="m1")
            nc.tensor.matmul(m1[:], v[:], ones, start=True, stop=True)
            m1sb = sm.tile([NCOL, 1], f32, tag="m1sb", name="m1sb")
            nc.vector.tensor_copy(m1sb[:], m1[:])
            m2 = pT.tile([1, 2], f32, tag="tiny", name="m2")
            nc.tensor.matmul(m2[:], m1sb[:], ssel, start=True, stop=True)
            res = sm.tile([1, 2], f32, tag="res", name="res")
            nc.vector.tensor_tensor(res[:], m2[:], offs, OP.add)
            dma_sp(out=o_out[:], in_=res[:])

    nc.compile()
    return nc


def get_program():
    if "nc" not in _COMPILED:
        _COMPILED["nc"] = _build_program()
    return _COMPILED["nc"]


# ----------------------------------------------------------------------------
# entry point
# ----------------------------------------------------------------------------

def _run(in_maps, trace=False):
    from concourse.bass_utils import run_bass_kernel_spmd
    nc = get_program()
    return run_bass_kernel_spmd(nc, in_maps, list(range(NCORES)), trace=trace)


def kernel(src_x, tgt_x, src_y, tgt_y):
    in_maps = _host_prep(src_x, tgt_x, src_y, tgt_y)
    if in_maps is None:
        return _numpy_fallback(src_x, tgt_x, src_y, tgt_y)
    br = _run(in_maps)
    total = np.zeros(2, np.float64)
    for res in br.results:
        total += res["out"].reshape(2).astype(np.float64)
    return total.astype(np.float32)


# revision 7
# speedup vs baseline: 1.3606x; 1.0467x over previous
"""CDD loss kernel for 8 Trainium2 NeuronCores (Bass/Tile, SPMD).

Math (validated vs reference in float32):
  ps is one-hot -> every (C,C,N,N) reference tensor collapses to per-class-
  block sums. Host sorts+pads src rows by class (CAP rows/class, pads are
  huge distinct sentinel vectors so exp(-dist/bw) underflows to exactly 0).
  The E_pp class-diagonal blocks have their diagonal zeroed on device, making
  each diagonal entry contribute exactly exp(0)=1 per bandwidth; the exact
  correction (5*CAP - 5*exp(-1e-5)*cs) is applied as a host-computed offset.
  g2 is symmetric -> T2 = T1^T, so inter = sum_{s!=t} 2*(T1-T3)/(C^2-C).

Distribution (SPMD, one program, per-core data):
  - host precomputes the feature transposes (bf16) and row norms (f32);
    the device only does Gram matmuls (bf16 in, f32 accum), the f32
    d2 assembly, and sqrt -> E in bf16
  - every core computes E rows for its class pair (rotation of the padded
    src rows makes "own" rows/cols sit at fixed offsets), partial
    S1 = Wown^T E_pp W, sst = Wown^T E_pt pt, stt = pt^T E_tt pt / 8
    via transpose-free two-stage small matmuls
  - one AllReduce of the packed [12,36] partials, issued as soon as the
    partials exist so it overlaps the gather/build work
  - E_own is written to DRAM in class-block layout so the T3 flat-block
    gather reads contiguous segments (2 DMAs); DMAs are spread across the
    SP and Activation HWDGE queues
  - exp-heavy sums run as single ACT instructions over flattened bf16
    broadcast tiles with per-partition scale and f32 accum_out; k2 runs
    as 5+3 per-pass (class,bandwidth) units per core (balanced across
    cores via a host-permuted bandwidth table folded into the ibg build)
  - per-core weighted reduce with host weight matrix -> [intra, inter]
    partials, host sums the 8 partials.
"""

import math
import numpy as np

C = 12
KN = 5
MU = 2
N = 384
D = 256
CAP = 64
R = C * CAP            # 768 padded src rows
NCORES = 8
NCOL = 20              # ACC columns: T1, T3, k1*5, k3*5, k2q0*5, k2q1*3
DIAG5 = 5.0 * math.exp(-1e-5)
I2 = 2.0 / (C * C - C)

# f32 misc pack column offsets ([128, MISCW])
O_WM = 0
O_ONES = 20
O_ONESR = 21          # row 0: ones [1,128]
O_RSCOL = 149         # [128,1]
O_RTCOL = 150         # [128,3]
O_RSROW = 153         # row 0: [1,768]
O_RTROW = 921         # row 0: [1,384]
O_EYE12 = 1305
O_OH2 = 1317
O_K2SEL = 1319
O_PERM65 = 1321       # rows 0-64
O_PW60 = 1386
O_PW10 = 1446
O_RDEN2 = 1456
O_RDENIN = 1468
O_SSEL = 1469         # rows 0-19
O_OFFS = 1471         # row 0
MISCW = 1473

# bf16 miscb pack column offsets ([128, MISCBW])
B_WR = 0              # 6 x [128,12]
B_PTB = 72            # 3 x [128,12]
B_WOWN = 108
B_DIAGM = 120
B_PCF = 184
B_PTR = 190           # row 0: ptr2a, ptr2b, ptr3a, ptr3b (4 x 384)
MISCBW = 1726

_COMPILED = {}


# ----------------------------------------------------------------------------
# host-side prep
# ----------------------------------------------------------------------------

def _host_prep(src_x, tgt_x, src_y, tgt_y):
    import ml_dtypes
    bf16 = ml_dtypes.bfloat16
    src_x = np.ascontiguousarray(np.asarray(src_x, dtype=np.float32))
    tgt_x = np.ascontiguousarray(np.asarray(tgt_x, dtype=np.float32))
    src_y = np.asarray(src_y).astype(np.int64)
    pt = np.ascontiguousarray(np.asarray(tgt_y, dtype=np.float32))

    counts = np.bincount(src_y, minlength=C)
    if counts.max() > CAP:
        return None  # caller falls back to numpy path

    perm = np.argsort(src_y, kind="stable")
    sx_pad = np.zeros((R, D), np.float32)
    W = np.zeros((R, C), np.float32)
    # pad sentinels: huge random-sign vectors. Pad-pad dot products are then
    # tiny relative to the norms (no catastrophic cancellation in d2), every
    # pad-involved distance is >= ~3e5 and exp(-dist/bw) underflows to 0.
    rng = np.random.default_rng(987654321)
    sgn = (rng.integers(0, 2, size=(R, D)).astype(np.float32) * 2.0 - 1.0)
    off = 0
    padidx = 0
    for c in range(C):
        idx = perm[off:off + counts[c]]
        sx_pad[c * CAP:c * CAP + counts[c]] = src_x[idx]
        W[c * CAP:c * CAP + counts[c], c] = 1.0
        for p in range(CAP - counts[c]):
            sx_pad[c * CAP + counts[c] + p, :] = 2.0e4 * sgn[padidx]
            padidx += 1
        off += counts[c]

    # round features to bf16 host-side; norms are computed from the rounded
    # values in f32 so the d2 diagonal cancels to ~0 on device
    sx_bf = sx_pad.astype(bf16)
    tx_bf = tgt_x.astype(bf16)
    sx_rf = sx_bf.astype(np.float32)
    tx_rf = tx_bf.astype(np.float32)

    txT_pack = np.zeros((128, 768), bf16)
    for k in range(2):
        txT_pack[:, k * N:(k + 1) * N] = tx_bf.T[k * 128:(k + 1) * 128, :]
    rtcol3 = np.zeros((128, 3), np.float32)
    for blk in range(3):
        rtcol3[:, blk] = (tx_rf[blk * 128:(blk + 1) * 128] ** 2).sum(1)
    rtrow = (tx_rf ** 2).sum(1)

    cs = counts.astype(np.float64)
    ct = pt.sum(0).astype(np.float64)
    pss = cs * cs
    ptt = ct * ct

    rden2 = (1.0 / (pss[:, None] + pss[None, :]
                    + 2.0 * cs[:, None] * cs[None, :])).astype(np.float32)
    rdenin = (1.0 / (pss + ptt + 2.0 * cs * ct)).astype(np.float32).reshape(C, 1)

    pw5 = np.array([-(float(MU) ** (k - KN // 2)) for k in range(KN)],
                   np.float32)
    pw60 = np.zeros((C, 60), np.float32)
    for k in range(KN):
        pw60[:, k * 12:(k + 1) * 12] = pw5[k]

    in_maps = []
    for r in range(NCORES):
        g = r % 6
        a, b = 2 * g, 2 * g + 1
        pp_active = r < 6
        roll = 2 * g * CAP

        sxr_bf = np.roll(sx_bf, -roll, axis=0)
        sxr_rf = np.roll(sx_rf, -roll, axis=0)
        sxT_pack = np.zeros((128, 1536), bf16)
        for k in range(2):
            sxT_pack[:, k * R:(k + 1) * R] = sxr_bf.T[k * 128:(k + 1) * 128, :]
        rscol = (sxr_rf[0:128] ** 2).sum(1)
        rsrow = (sxr_rf ** 2).sum(1)

        wr = np.roll(W, -roll, axis=0)
        wown = wr[0:128].copy() if pp_active else np.zeros((128, C), np.float32)

        oh2 = np.zeros((C, 2), np.float32)
        oh2[a, 0] = 1.0
        oh2[b, 1] = 1.0

        # k2 split: q0 = class r with all 5 bandwidths; q1 = class 8+(r%4)
        # with bandwidths {0,1,2} on cores 0-3 and {3,4,dup} on cores 4-7.
        c_q0 = r
        c_q1 = 8 + (r % 4)
        kq1 = [0, 1, 2] if r < 4 else [3, 4]
        k2sel = np.zeros((C, 2), np.float32)
        k2sel[c_q0, 0] = 1.0
        k2sel[c_q1, 1] = 1.0
        pw10 = np.zeros((C, 10), np.float32)
        pw10[:, 0:5] = pw5[None, :]
        for j in range(5):
            pw10[:, 5 + j] = pw5[kq1[j]] if j < len(kq1) else pw5[0]

        ptrow2 = np.zeros((2, N), np.float32)
        ptcolf = np.zeros((128, 6), np.float32)
        for q, c in enumerate((c_q0, c_q1)):
            ptrow2[q] = pt[:, c]
            for blk in range(3):
                ptcolf[:, q * 3 + blk] = pt[blk * 128:(blk + 1) * 128, c]

        # perm65: sclT3[j] = ibg[cls, perm(j)] via matmul(lhsT=perm65, rhs=selcol)
        # row layout j = k*12 + t; source col = k*12 + rot(t), rot(t)=(2g+t)%12
        perm65 = np.zeros((65, 65), np.float32)
        for k in range(KN):
            for t in range(12):
                perm65[k * 12 + ((2 * g + t) % 12), k * 12 + t] = 1.0
        for j in range(60, 65):
            perm65[j, j] = 1.0

        wm = np.zeros((128, NCOL), np.float32)
        if pp_active:
            for h, cls in ((0, a), (1, b)):
                for k in range(KN):
                    for t in range(12):
                        if t != cls:
                            wm[h * 64 + k * 12 + t, 0] = I2 / pss[cls]
                        rt_ = (2 * g + t) % 12
                        if rt_ != cls:
                            wm[h * 64 + k * 12 + t, 1] = \
                                -I2 / (cs[cls] * cs[rt_])
                for k in range(KN):
                    wm[h * CAP:(h + 1) * CAP, 2 + k] = 1.0 / (C * pss[cls])
                    wm[h * CAP:(h + 1) * CAP, 7 + k] = \
                        -2.0 / (C * cs[cls] * ct[cls])
        wm[:, 12:17] = 1.0 / (C * ptt[c_q0])
        for j in range(len(kq1)):
            wm[:, 17 + j] = 1.0 / (C * ptt[c_q1])

        ssel = np.zeros((NCOL, 2), np.float32)
        ssel[2:NCOL, 0] = 1.0   # intra cols: k1, k3, k2
        ssel[0:2, 1] = 1.0      # inter cols: T1, T3

        offs = np.zeros((1, 2), np.float32)
        if r == 0:
            corr = 5.0 * CAP - DIAG5 * cs
            offs[0, 0] = -(corr / pss / C).sum()
            offs[0, 1] = -((C - 1) * corr * I2 / pss).sum()

        misc = np.zeros((128, MISCW), np.float32)
        misc[:, O_WM:O_WM + NCOL] = wm
        misc[:, O_ONES] = 1.0
        misc[0, O_ONESR:O_ONESR + 128] = 1.0
        misc[:, O_RSCOL] = rscol
        misc[:, O_RTCOL:O_RTCOL + 3] = rtcol3
        misc[0, O_RSROW:O_RSROW + R] = rsrow
        misc[0, O_RTROW:O_RTROW + N] = rtrow
        misc[0:12, O_EYE12:O_EYE12 + 12] = np.eye(C, dtype=np.float32)
        misc[0:12, O_OH2:O_OH2 + 2] = oh2
        misc[0:12, O_K2SEL:O_K2SEL + 2] = k2sel
        misc[0:65, O_PERM65:O_PERM65 + 65] = perm65
        misc[0:12, O_PW60:O_PW60 + 60] = pw60
        misc[0:12, O_PW10:O_PW10 + 10] = pw10
        misc[0:12, O_RDEN2:O_RDEN2 + 12] = rden2
        misc[0:12, O_RDENIN:O_RDENIN + 1] = rdenin
        misc[0:NCOL, O_SSEL:O_SSEL + 2] = ssel
        misc[0, O_OFFS:O_OFFS + 2] = offs[0]

        miscb = np.zeros((128, MISCBW), np.float32)
        for m in range(6):
            miscb[:, B_WR + m * 12:B_WR + (m + 1) * 12] = \
                wr[m * 128:(m + 1) * 128]
        for m in range(3):
            miscb[:, B_PTB + m * 12:B_PTB + (m + 1) * 12] = \
                pt[m * 128:(m + 1) * 128]
        miscb[:, B_WOWN:B_WOWN + 12] = wown
        miscb[0:CAP, B_DIAGM:B_DIAGM + CAP] = 1.0 - np.eye(CAP)
        miscb[CAP:128, B_DIAGM:B_DIAGM + CAP] = 1.0 - np.eye(CAP)
        miscb[:, B_PCF:B_PCF + 6] = ptcolf
        miscb[0, B_PTR:B_PTR + N] = ptrow2[0]
        miscb[0, B_PTR + N:B_PTR + 2 * N] = ptrow2[1]
        miscb[0, B_PTR + 2 * N:B_PTR + 3 * N] = pt[:, a]
        miscb[0, B_PTR + 3 * N:B_PTR + 4 * N] = pt[:, b]

        in_maps.append({
            "sxT": sxT_pack,
            "txT": txT_pack,
            "misc": np.ascontiguousarray(misc),
            "miscb": np.ascontiguousarray(miscb.astype(bf16)),
        })
    return in_maps


def _numpy_fallback(src_x, tgt_x, src_y, tgt_y):
    f = np.float32
    src_x = np.asarray(src_x, f)
    tgt_x = np.asarray(tgt_x, f)
    src_y = np.asarray(src_y).astype(np.int64)
    pt = np.asarray(tgt_y, f)
    ps = np.eye(C, dtype=f)[src_y]

    def cdist(a, bb):
        d2 = (a * a).sum(1)[:, None] + (bb * bb).sum(1)[None, :] - 2.0 * (a @ bb.T)
        return np.sqrt(np.maximum(d2, 0.0))

    def kern(dist, g):
        acc = 0.0
        for i in range(KN):
            bw = np.maximum(np.asarray(g) * (MU ** (i - KN // 2)), 1e-5)
            acc = acc + np.exp(-np.clip(dist / bw, 1e-5, 1e5))
        return acc

    E_ss = cdist(src_x, src_x); E_tt = cdist(tgt_x, tgt_x); E_st = cdist(src_x, tgt_x)
    sss = np.einsum('ic,ij,jc->c', ps, E_ss, ps)
    stt = np.einsum('ic,ij,jc->c', pt, E_tt, pt)
    sst = np.einsum('is,ij,jt->st', ps, E_st, pt)
    cs = ps.sum(0); ct = pt.sum(0)
    pss = cs * cs; ptt = ct * ct; pstd = cs * ct
    g_in = (sss + stt + 2 * np.diagonal(sst)) / (pss + ptt + 2 * pstd)
    Pss = ps.T[:, :, None] * ps.T[:, None, :]
    Ptt = pt.T[:, :, None] * pt.T[:, None, :]
    Pst = ps.T[:, :, None] * pt.T[:, None, :]
    k1 = (kern(E_ss[None] * Pss, g_in[:, None, None]) * Pss).sum((-2, -1)) / pss
    k2 = (kern(E_tt[None] * Ptt, g_in[:, None, None]) * Ptt).sum((-2, -1)) / ptt
    k3 = (kern(E_st[None] * Pst, g_in[:, None, None]) * Pst).sum((-2, -1)) / pstd
    intra = (k1 + k2 - 2 * k3).sum() / C
    sst_s = np.einsum('is,ij,jt->st', ps, E_ss, ps)
    g2 = (sss[:, None] + sss[None, :] + 2 * sst_s) / (
        pss[:, None] + pss[None, :] + 2 * cs[:, None] * cs[None, :])
    T1 = np.zeros((C, C), f); T3 = np.zeros((C, C), f)
    for s in range(C):
        ms = ps[:, s].astype(bool)
        for t in range(C):
            mt = ps[:, t].astype(bool)
            T1[s, t] = kern(E_ss[np.ix_(ms, ms)], g2[s, t]).sum() / pss[s]
            T3[s, t] = kern(E_ss[np.ix_(ms, mt)], g2[s, t]).sum() / (cs[s] * cs[t])
    inter = ((2 * T1 - 2 * T3) * (1 - np.eye(C))).sum() / (C * C - C)
    return np.array([intra, inter], np.float32)


# ----------------------------------------------------------------------------
# device program
# ----------------------------------------------------------------------------

def _build_program():
    import concourse.bass as bass
    import concourse.tile as tile
    from concourse import bacc, mybir

    f32 = mybir.dt.float32
    bf = mybir.dt.bfloat16
    AF = mybir.ActivationFunctionType
    OP = mybir.AluOpType

    nc = bacc.Bacc("TRN2", target_bir_lowering=False, debug=False,
                   num_devices=NCORES)

    i_sxT = nc.dram_tensor("sxT", [128, 2 * R], bf, kind="ExternalInput").ap()
    i_txT = nc.dram_tensor("txT", [128, 2 * N], bf, kind="ExternalInput").ap()
    i_misc = nc.dram_tensor("misc", [128, MISCW], f32, kind="ExternalInput").ap()
    i_miscb = nc.dram_tensor("miscb", [128, MISCBW], bf,
                             kind="ExternalInput").ap()

    o_out = nc.dram_tensor("out", [1, 2], f32, kind="ExternalOutput").ap()

    with tile.TileContext(nc) as tc:
        with (
            tc.tile_pool(name="io", bufs=1) as io,
            tc.tile_pool(name="big", bufs=1) as big,
            tc.tile_pool(name="scr", bufs=2) as scr,
            tc.tile_pool(name="sm", bufs=1) as sm,
            tc.tile_pool(name="pG", bufs=2, space="PSUM") as pG,
            tc.tile_pool(name="p1", bufs=2, space="PSUM") as p1,
            tc.tile_pool(name="pT", bufs=2, space="PSUM") as pT,
            tc.tile_pool(name="pS", bufs=1, space="PSUM") as pS,
            tc.tile_pool(name="dram", bufs=1, space="DRAM") as dpool,
        ):
            dma_sp = nc.sync.dma_start
            dma_act = nc.scalar.dma_start

            # ---------------- input loads: 4 big DMAs ----------------
            sxT = io.tile([128, 2 * R], bf, tag="sxT", name="sxT")
            dma_sp(out=sxT[:], in_=i_sxT[:])
            txT = io.tile([128, 2 * N], bf, tag="txT", name="txT")
            dma_act(out=txT[:], in_=i_txT[:])
            misc = io.tile([128, MISCW], f32, tag="misc", name="misc")
            dma_sp(out=misc[:], in_=i_misc[:])
            miscb = io.tile([128, MISCBW], bf, tag="miscb", name="miscb")
            dma_act(out=miscb[:], in_=i_miscb[:])

            wm = misc[:, O_WM:O_WM + NCOL]
            ones = misc[:, O_ONES:O_ONES + 1]
            onesr = misc[0:1, O_ONESR:O_ONESR + 128]
            rscol = misc[:, O_RSCOL:O_RSCOL + 1]
            rtcol = misc[:, O_RTCOL:O_RTCOL + 3]
            rsrow = misc[0:1, O_RSROW:O_RSROW + R]
            rtrow = misc[0:1, O_RTROW:O_RTROW + N]
            eye12 = misc[0:12, O_EYE12:O_EYE12 + 12]
            oh2 = misc[0:12, O_OH2:O_OH2 + 2]
            k2sel = misc[0:12, O_K2SEL:O_K2SEL + 2]
            perm65 = misc[0:65, O_PERM65:O_PERM65 + 65]
            pw60 = misc[0:12, O_PW60:O_PW60 + 60]
            pw10 = misc[0:12, O_PW10:O_PW10 + 10]
            rden2 = misc[0:12, O_RDEN2:O_RDEN2 + 12]
            rdenin = misc[0:12, O_RDENIN:O_RDENIN + 1]
            ssel = misc[0:NCOL, O_SSEL:O_SSEL + 2]
            offs = misc[0:1, O_OFFS:O_OFFS + 2]

            wrb = [miscb[:, B_WR + m * 12:B_WR + (m + 1) * 12] for m in range(6)]
            ptb = [miscb[:, B_PTB + m * 12:B_PTB + (m + 1) * 12]
                   for m in range(3)]
            wown = miscb[:, B_WOWN:B_WOWN + 12]
            diagm = miscb[:, B_DIAGM:B_DIAGM + CAP]
            pcf = miscb[:, B_PCF:B_PCF + 6]
            ptr2 = [miscb[0:1, B_PTR + q * N:B_PTR + (q + 1) * N]
                    for q in range(2)]
            ptr3 = [miscb[0:1, B_PTR + (q + 2) * N:B_PTR + (q + 3) * N]
                    for q in range(2)]

            # ---------------- gpsimd broadcasts (all pre-collective) --------
            rsrowb = big.tile([128, R], f32, tag="rsrowb", name="rsrowb")
            rtrowb = big.tile([128, N], f32, tag="rtrowb", name="rtrowb")
            nc.gpsimd.partition_broadcast(rsrowb[:], rsrow)
            nc.gpsimd.partition_broadcast(rtrowb[:], rtrow)
            ptrow2b = [big.tile([128, N], bf, tag=f"ptrow2b{q}",
                                name=f"ptrow2b{q}") for q in range(2)]
            nc.gpsimd.partition_broadcast(ptrow2b[0][:], ptr2[0])
            nc.gpsimd.partition_broadcast(ptrow2b[1][:], ptr2[1])
            ptw3 = big.tile([128, N], bf, tag="ptw3", name="ptw3")
            ptw3t = big.tile([128, N], bf, tag="ptw3t", name="ptw3t")
            nc.gpsimd.partition_broadcast(ptw3[:], ptr3[0])
            nc.gpsimd.partition_broadcast(ptw3t[:], ptr3[1])
            nc.vector.tensor_copy(ptw3[CAP:128, :], ptw3t[CAP:128, :])

            # ---------------- E matrices (bf16 in/out, f32 d2) ----------------
            sxTk = [sxT[:, 0:R], sxT[:, R:2 * R]]
            txTk = [txT[:, 0:N], txT[:, N:2 * N]]

            def emit_E(dst, lhsT_k, lhs_lo, rhs_k, n_cols, rcol_ap, rowb):
                done = 0
                while done < n_cols:
                    nchunk = min(512, n_cols - done)
                    gp = pG.tile([128, 512], f32, tag="G", name="gp")
                    for k in range(2):
                        nc.tensor.matmul(
                            gp[:, :nchunk],
                            lhsT_k[k][:, lhs_lo:lhs_lo + 128],
                            rhs_k[k][:, done:done + nchunk],
                            start=(k == 0), stop=(k == 1))
                    t1_ = scr.tile([128, 512], f32, tag="d2scr", name="d2s")
                    nc.vector.scalar_tensor_tensor(
                        out=t1_[:, :nchunk], in0=gp[:, :nchunk], scalar=-2.0,
                        in1=rowb[:, done:done + nchunk],
                        op0=OP.mult, op1=OP.add)
                    nc.vector.tensor_scalar(
                        t1_[:, :nchunk], t1_[:, :nchunk],
                        rcol_ap, 0.0, OP.add, OP.max)
                    nc.scalar.activation(dst[:, done:done + nchunk],
                                         t1_[:, :nchunk], AF.Sqrt)
                    done += nchunk

            E_own = big.tile([128, R], bf, tag="E_own", name="E_own")
            emit_E(E_own, sxTk, 0, sxTk, R, rscol, rsrowb)

            E_ttf = big.tile([128, 3 * N], bf, tag="E_ttf", name="E_ttf")
            for blk in range(3):
                emit_E(E_ttf[:, blk * N:(blk + 1) * N], txTk, blk * 128, txTk,
                       N, rtcol[:, blk:blk + 1], rtrowb)

            E_pt = big.tile([128, N], bf, tag="E_pt", name="E_pt")
            emit_E(E_pt, sxTk, 0, txTk, N, rscol, rtrowb)

            # ---------------- partial sums + collective (EARLY) --------------
            # two-stage transpose-free chains:
            #   stage1[j,s] = (E^T w)[j,s] (128-row col-blocks, bf16 copies)
            #   stage2 accumulates stage1^T w2 -> [12,12] f32 PSUM
            part = sm.tile([C, 36], f32, tag="part", name="part")

            s1ps = pS.tile([C, C], f32, tag="S", name="s1ps")
            for sub in range(6):
                pp = p1.tile([128, C], f32, tag="p1", name="pp")
                nc.tensor.matmul(pp[:], E_own[:, sub * 128:(sub + 1) * 128],
                                 wown, start=True, stop=True)
                cb = scr.tile([128, C], bf, tag="cbs", name="cb")
                nc.vector.tensor_copy(cb[:], pp[:])
                nc.tensor.matmul(s1ps[:], cb[:], wrb[sub],
                                 start=(sub == 0), stop=(sub == 5))
            nc.vector.tensor_copy(part[:, 0:12], s1ps[:])

            stps = pS.tile([C, C], f32, tag="S", name="stps")
            for sub in range(3):
                pp = p1.tile([128, C], f32, tag="p1", name="pp")
                for blk in range(3):
                    nc.tensor.matmul(
                        pp[:],
                        E_ttf[:, blk * N + sub * 128:blk * N + (sub + 1) * 128],
                        ptb[blk], start=(blk == 0), stop=(blk == 2))
                cb = scr.tile([128, C], bf, tag="cbs", name="cb")
                nc.vector.tensor_copy(cb[:], pp[:])
                nc.tensor.matmul(stps[:], cb[:], ptb[sub],
                                 start=(sub == 0), stop=(sub == 2))
            nc.vector.tensor_scalar_mul(part[:, 12:24], stps[:], 1.0 / NCORES)

            ssps = pS.tile([C, C], f32, tag="S", name="ssps")
            for sub in range(3):
                pp = p1.tile([128, C], f32, tag="p1", name="pp")
                nc.tensor.matmul(pp[:], E_pt[:, sub * 128:(sub + 1) * 128],
                                 wown, start=True, stop=True)
                cb = scr.tile([128, C], bf, tag="cbs", name="cb")
                nc.vector.tensor_copy(cb[:], pp[:])
                nc.tensor.matmul(ssps[:], cb[:], ptb[sub],
                                 start=(sub == 0), stop=(sub == 2))
            nc.vector.tensor_copy(part[:, 24:36], ssps[:])

            d_ccin = dpool.tile([C, 36], f32, tag="d_ccin", name="d_ccin")
            d_ccout = dpool.tile([C, 36], f32, tag="d_ccout", name="d_ccout")
            dma_sp(out=d_ccin[:], in_=part[:])
            nc.gpsimd.collective_compute(
                "AllReduce", mybir.AluOpType.add,
                replica_groups=[list(range(NCORES))],
                ins=[d_ccin.opt()], outs=[d_ccout.opt()])
            sred = sm.tile([C, 36], f32, tag="sred", name="sred")
            dma_sp(out=sred[:], in_=d_ccout[:])

            # ---------------- gather/build work (overlaps the collective) ----
            E_diag = big.tile([128, CAP], bf, tag="E_diag", name="E_diag")
            nc.vector.tensor_tensor(E_diag[0:CAP, :], E_own[0:CAP, 0:CAP],
                                    diagm[0:CAP, :], OP.mult)
            nc.vector.tensor_tensor(E_diag[CAP:128, :],
                                    E_own[CAP:128, CAP:128],
                                    diagm[CAP:128, :], OP.mult)

            d_ed = dpool.tile([128, CAP], bf, tag="d_ed", name="d_ed")
            dma_sp(out=d_ed[:], in_=E_diag[:])
            # E_own -> DRAM in class-block layout [t][row][col] so each
            # (half, t) block is one contiguous segment
            d_eob = dpool.tile([C, 128 * CAP], bf, tag="d_eob", name="d_eob")
            dst_eob = bass.AP(tensor=d_eob.tensor, offset=0,
                              ap=[[CAP, 128], [128 * CAP, 12], [1, CAP]])
            dma_act(out=dst_eob, in_=E_own[:])

            t1src = big.tile([128, CAP * CAP], bf, tag="t1src", name="t1src")
            for h in range(2):
                ap_in = bass.AP(tensor=d_ed.tensor, offset=h * CAP * CAP,
                                ap=[[0, 64], [1, CAP * CAP]])
                dma_sp(out=t1src[h * 64:(h + 1) * 64, :], in_=ap_in)

            # T3 flat blocks: partition h*64 + k*12 + t <- block (h, t)
            t3src = big.tile([128, CAP * CAP], bf, tag="t3src", name="t3src")
            for h in range(2):
                ap_in = bass.AP(tensor=d_eob.tensor, offset=h * CAP * CAP,
                                ap=[[0, 5], [128 * CAP, 12], [1, CAP * CAP]])
                dma_act(out=t3src[h * 64:h * 64 + 60, :], in_=ap_in)
                ap_pad = bass.AP(tensor=d_eob.tensor, offset=h * CAP * CAP,
                                 ap=[[0, 4], [1, CAP * CAP]])
                dma_act(out=t3src[h * 64 + 60:h * 64 + 64, :], in_=ap_pad)

            # ---------------- k2 / k3 static builds ----------------
            k2P = []
            k2D = []
            for q in range(2):
                P = big.tile([128, 3 * N], bf, tag=f"k2P{q}", name=f"k2P{q}")
                colap = bass.AP(tensor=pcf.tensor,
                                offset=pcf.offset + q * 3,
                                ap=[list(pcf.ap[0]), [1, 3], [0, N]])
                rowap = bass.AP(tensor=ptrow2b[q].tensor,
                                offset=ptrow2b[q].offset,
                                ap=[list(ptrow2b[q].ap[0]), [0, 3], [1, N]])
                nc.vector.tensor_tensor(P[:], colap, rowap, OP.mult)
                Dt = big.tile([128, 3 * N], bf, tag=f"k2D{q}", name=f"k2D{q}")
                nc.vector.tensor_tensor(Dt[:], E_ttf[:], P[:], OP.mult)
                k2P.append(P)
                k2D.append(Dt)

            k3D = big.tile([128, N], bf, tag="k3D", name="k3D")
            nc.vector.tensor_tensor(k3D[:], E_pt[:], ptw3[:], OP.mult)

            acc = big.tile([128, NCOL], f32, tag="acc", name="acc")
            nc.vector.memset(acc[:], 0.0)
            sclT1 = sm.tile([128, 1], f32, tag="sclT1", name="sclT1")
            sclT3 = sm.tile([128, 1], f32, tag="sclT3", name="sclT3")
            nc.vector.memset(sclT1[:], 0.0)
            nc.vector.memset(sclT3[:], 0.0)

            # ---------------- gammas (post-collective) ----------------
            S1 = sred[:, 0:12]
            sttM = sred[:, 12:24]
            sstM = sred[:, 24:36]

            def diag_col(mat, nm):
                s_ = scr.tile([C, C], f32, tag="diagscr", name="dsc")
                col = sm.tile([C, 1], f32, tag=nm, name=nm)
                nc.vector.tensor_tensor(s_[:], mat, eye12, OP.mult)
                nc.vector.reduce_sum(out=col[:], in_=s_[:],
                                     axis=mybir.AxisListType.X)
                return col

            ssscol = diag_col(S1, "ssscol")
            sttcol = diag_col(sttM, "sttcol")
            sstdcol = diag_col(sstM, "sstdcol")

            gin = sm.tile([C, 1], f32, tag="gin", name="gin")
            nc.vector.scalar_tensor_tensor(out=gin[:], in0=sstdcol[:], scalar=2.0,
                                           in1=sttcol[:], op0=OP.mult, op1=OP.add)
            nc.vector.tensor_tensor(gin[:], gin[:], ssscol[:], OP.add)
            nc.vector.tensor_tensor(gin[:], gin[:], rdenin, OP.mult)

            ssst = pT.tile([1, C], f32, tag="tiny", name="ssst")
            nc.tensor.transpose(ssst[:], ssscol[:], eye12)
            ssstsb = sm.tile([1, C], f32, tag="ssstsb", name="ssstsb")
            nc.vector.tensor_copy(ssstsb[:], ssst[:])
            # rank-1 broadcast via TensorE (faster than gpsimd on the
            # post-collective critical path)
            ps12 = pT.tile([C, C], f32, tag="tiny", name="ps12")
            nc.tensor.matmul(ps12[:], onesr[0:1, 0:12], ssstsb[:],
                             start=True, stop=True)
            sssrowb = sm.tile([C, C], f32, tag="sssrowb", name="sssrowb")
            nc.vector.tensor_copy(sssrowb[:], ps12[:])
            g2 = sm.tile([C, C], f32, tag="g2", name="g2")
            nc.vector.tensor_scalar(g2[:], S1, 2.0, None, OP.mult)
            nc.vector.tensor_tensor(g2[:], g2[:], sssrowb[:], OP.add)
            nc.vector.tensor_scalar(g2[:], g2[:], ssscol[:], None, OP.add)
            nc.vector.tensor_tensor(g2[:], g2[:], rden2, OP.mult)

            # IBG [12, 70] = -1/bw : cols 0-59 from g2 (k-major),
            # 60-64 from gin (k-order), 65-69 from gin (host-permuted for q1)
            ibg0 = sm.tile([C, 70], f32, tag="ibg0", name="ibg0")
            g2ap = g2[:]
            g2exp = bass.AP(tensor=g2ap.tensor, offset=g2ap.offset,
                            ap=[list(g2ap.ap[0]), [0, 5], [1, 12]])
            nc.vector.tensor_tensor(ibg0[:, 0:60], g2exp, pw60, OP.mult)
            ginap = gin[:]
            ginexp = bass.AP(tensor=ginap.tensor, offset=ginap.offset,
                             ap=[list(ginap.ap[0]), [0, 10]])
            nc.vector.tensor_tensor(ibg0[:, 60:70], ginexp, pw10, OP.mult)
            nc.vector.tensor_scalar(ibg0[:], ibg0[:], -1e-5, None, OP.min)
            ibg = sm.tile([C, 70], f32, tag="ibg", name="ibg")
            nc.vector.reciprocal(ibg[:], ibg0[:])

            # per-half scale vectors
            negk1 = sm.tile([128, 5], f32, tag="negk1", name="negk1")
            for h in range(2):
                ps_ = pT.tile([1, 65], f32, tag="tiny", name="psel")
                nc.tensor.matmul(ps_[:], oh2[:, h:h + 1], ibg[:, 0:65],
                                 start=True, stop=True)
                selsb = sm.tile([1, 65], f32, tag=f"sel{h}", name=f"sel{h}")
                nc.vector.tensor_copy(selsb[:], ps_[:])
                tp_ = pT.tile([65, 1], f32, tag="tiny", name="tsel")
                nc.tensor.transpose(tp_[:], selsb[:], ones[0:1, :])
                tpsb = scr.tile([65, 1], f32, tag="tselsb", name="tpsb")
                nc.vector.tensor_copy(tpsb[:], tp_[:])
                nc.vector.tensor_copy(sclT1[h * 64:h * 64 + 60, :], tpsb[0:60, :])
                p3 = pT.tile([65, 1], f32, tag="tiny", name="p3")
                nc.tensor.matmul(p3[:], perm65, tpsb[:], start=True, stop=True)
                p3sb = scr.tile([65, 1], f32, tag="p3sb", name="p3sb")
                nc.vector.tensor_copy(p3sb[:], p3[:])
                nc.vector.tensor_copy(sclT3[h * 64:h * 64 + 60, :], p3sb[0:60, :])
                pnk = pT.tile([128, 5], f32, tag="tiny", name="pnk")
                nc.tensor.matmul(pnk[:], onesr, selsb[0:1, 60:65],
                                 start=True, stop=True)
                if h == 0:
                    nc.vector.tensor_copy(negk1[0:CAP, :], pnk[0:CAP, :])
                else:
                    nc.vector.tensor_copy(negk1[CAP:128, :], pnk[CAP:128, :])

            negb = []
            for q in range(2):
                k2sc = pT.tile([1, 5], f32, tag="tiny", name="k2sc")
                nc.tensor.matmul(k2sc[:], k2sel[:, q:q + 1],
                                 ibg[:, 60 + 5 * q:65 + 5 * q],
                                 start=True, stop=True)
                k2scsb = sm.tile([1, 5], f32, tag=f"k2scsb{q}", name=f"k2scsb{q}")
                nc.vector.tensor_copy(k2scsb[:], k2sc[:])
                pnb = pT.tile([128, 5], f32, tag="tiny", name="pnb")
                nc.tensor.matmul(pnb[:], onesr, k2scsb[:], start=True, stop=True)
                nb = sm.tile([128, 5], f32, tag=f"negb{q}", name=f"negb{q}")
                nc.vector.tensor_copy(nb[:], pnb[:])
                negb.append(nb)

            # ---------------- exp passes ----------------
            nc.scalar.activation(t1src[:], t1src[:], AF.Exp, scale=sclT1[:],
                                 accum_out=acc[:, 0:1])
            nc.scalar.activation(t3src[:], t3src[:], AF.Exp, scale=sclT3[:],
                                 accum_out=acc[:, 1:2])

            for k in range(KN):
                sk = scr.tile([128, CAP], bf, tag="k1scr", name="sk1")
                nc.scalar.activation(sk[:], E_diag[:], AF.Exp,
                                     scale=negk1[:, k:k + 1],
                                     accum_out=acc[:, 2 + k:3 + k])

            for k in range(KN):
                ek = scr.tile([128, N], bf, tag="k3e", name="ek3")
                nc.scalar.activation(ek[:], k3D[:], AF.Exp,
                                     scale=negk1[:, k:k + 1])
                sk = scr.tile([128, N], bf, tag="k3scr", name="sk3")
                nc.vector.scalar_tensor_tensor(
                    out=sk[:], in0=ek[:], scalar=1.0, in1=ptw3[:],
                    op0=OP.mult, op1=OP.mult,
                    accum_out=acc[:, 7 + k:8 + k])

            for q in range(2):
                npass = 5 if q == 0 else 3
                for j in range(npass):
                    ek = scr.tile([128, 3 * N], bf, tag="k2e", name="ek2")
                    nc.scalar.activation(ek[:], k2D[q][:], AF.Exp,
                                         scale=negb[q][:, j:j + 1])
                    sk = scr.tile([128, 3 * N], bf, tag="k2scr", name="sk2")
                    col = 12 + 5 * q + j
                    nc.vector.scalar_tensor_tensor(
                        out=sk[:], in0=ek[:], scalar=1.0, in1=k2P[q][:],
                        op0=OP.mult, op1=OP.mult,
                        accum_out=acc[:, col:col + 1])

            # ---------------- final weighted reduce ----------------
            v = big.tile([128, NCOL], f32, tag="v", name="v")
            nc.vector.tensor_tensor(v[:], acc[:], wm, OP.mult)
            m1 = pT.tile([NCOL, 1], f32, tag="tiny", name="m1")
            nc.tensor.matmul(m1[:], v[:], ones, start=True, stop=True)
            m1sb = sm.tile([NCOL, 1], f32, tag="m1sb", name="m1sb")
            nc.vector.tensor_copy(m1sb[:], m1[:])
            m2 = pT.tile([1, 2], f32, tag="tiny", name="m2")
            nc.tensor.matmul(m2[:], m1sb[:], ssel, start=True, stop=True)
            res = sm.tile([1, 2], f32, tag="res", name="res")
            nc.vector.tensor_tensor(res[:], m2[:], offs, OP.add)
            dma_sp(out=o_out[:], in_=res[:])

    nc.compile()
    return nc


def get_program():
    if "nc" not in _COMPILED:
        _COMPILED["nc"] = _build_program()
    return _COMPILED["nc"]


# ----------------------------------------------------------------------------
# entry point
# ----------------------------------------------------------------------------

def _run(in_maps, trace=False):
    from concourse.bass_utils import run_bass_kernel_spmd
    nc = get_program()
    return run_bass_kernel_spmd(nc, in_maps, list(range(NCORES)), trace=trace)


def kernel(src_x, tgt_x, src_y, tgt_y):
    in_maps = _host_prep(src_x, tgt_x, src_y, tgt_y)
    if in_maps is None:
        return _numpy_fallback(src_x, tgt_x, src_y, tgt_y)
    br = _run(in_maps)
    total = np.zeros(2, np.float64)
    for res in br.results:
        total += res["out"].reshape(2).astype(np.float64)
    return total.astype(np.float32)


# revision 10
# speedup vs baseline: 2.1222x; 1.5597x over previous
"""CDD loss kernel for 8 Trainium2 NeuronCores (Bass/Tile, SPMD).

Math (validated vs reference in float32):
  ps is one-hot -> every (C,C,N,N) reference tensor collapses to per-class-
  block sums. Host sorts+pads src rows by class (CAP rows/class, pads are
  huge distinct sentinel vectors so exp(-dist/bw) underflows to exactly 0).
  The E_pp class-diagonal blocks have their diagonal zeroed on device, making
  each diagonal entry contribute exactly exp(0)=1 per bandwidth; the exact
  correction (5*CAP - 5*exp(-1e-5)*cs) is applied as a host-computed offset.
  g2 is symmetric -> T2 = T1^T, so inter = sum_{s!=t} 2*(T1-T3)/(C^2-C).

Distribution (SPMD, one program, per-core data):
  - NO collective: an 8-core AllReduce has a ~95us floor in this
    environment, far more than recomputing the [12,12] global sums
    locally. Every core computes the FULL (rotated) E_ss (6 slabs),
    E_tt, and E_pt (6 slabs) in bf16 and derives the gamma sums
    S1 = W^T E_ss W, stt = pt^T E_tt pt, sst = W^T E_st pt itself
    via transpose-free two-stage small matmuls.
  - host precomputes the feature transposes (bf16) and row norms (f32);
    the device does Gram matmuls (bf16 in, f32 accum), one f32 STT for
    d2-partial, and sqrt(x + rownorm_eps) via ACT bias -> E in bf16.
    A +0.5 epsilon in the per-partition norms keeps sqrt's argument
    positive under bf16 rounding (E error ~0.25/E, negligible).
  - exp work stays sharded: each core exponentiates only its own class
    pair's T1/T3/k1/k3 blocks (flat broadcast gathers through DRAM in
    class-block layout, contiguous segments, SP+Act HWDGE queues) and
    its 5+3 (class,bandwidth) k2 units (balanced via a host-permuted
    bandwidth table folded into the ibg build).
  - per-core weighted reduce with host weight matrix -> [intra, inter]
    partials, host sums the 8 partials.
"""

import math
import numpy as np

C = 12
KN = 5
MU = 2
N = 384
D = 256
CAP = 64
R = C * CAP            # 768 padded src rows
NCORES = 8
NCOL = 20              # ACC columns: T1, T3, k1*5, k3*5, k2q0*5, k2q1*3
DIAG5 = 5.0 * math.exp(-1e-5)
I2 = 2.0 / (C * C - C)
EPS = 0.5              # d2 positivity epsilon folded into the col norms

# f32 misc pack column offsets ([128, MISCW])
O_WM = 0
O_ONES = 20
O_ONESR = 21          # row 0: ones [1,128]
O_RSCOL6 = 149        # [128,6] per-slab src col norms (+EPS)
O_RTCOL = 155         # [128,3] tgt col norms (+EPS)
O_RSROW = 158         # row 0: [1,768]
O_RTROW = 926         # row 0: [1,384]
O_EYE12 = 1310
O_OH2 = 1322
O_K2SEL = 1324
O_PERM65 = 1326       # rows 0-64
O_PW60 = 1391
O_PW10 = 1451
O_RDEN2 = 1461
O_RDENIN = 1473
O_SSEL = 1474         # rows 0-19
O_OFFS = 1476         # row 0
MISCW = 1478

# bf16 miscb pack column offsets ([128, MISCBW])
B_WR = 0              # 6 x [128,12]
B_PTB = 72            # 3 x [128,12]
B_DIAGM = 108
B_PCF = 172
B_PTR = 178           # row 0: ptr2a, ptr2b, ptr3a, ptr3b (4 x 384)
MISCBW = 1714

_COMPILED = {}


# ----------------------------------------------------------------------------
# host-side prep
# ----------------------------------------------------------------------------

def _host_prep(src_x, tgt_x, src_y, tgt_y):
    import ml_dtypes
    bf16 = ml_dtypes.bfloat16
    src_x = np.ascontiguousarray(np.asarray(src_x, dtype=np.float32))
    tgt_x = np.ascontiguousarray(np.asarray(tgt_x, dtype=np.float32))
    src_y = np.asarray(src_y).astype(np.int64)
    pt = np.ascontiguousarray(np.asarray(tgt_y, dtype=np.float32))

    counts = np.bincount(src_y, minlength=C)
    if counts.max() > CAP:
        return None  # caller falls back to numpy path

    perm = np.argsort(src_y, kind="stable")
    sx_pad = np.zeros((R, D), np.float32)
    W = np.zeros((R, C), np.float32)
    # pad sentinels: huge random-sign vectors. Pad-pad dot products are then
    # tiny relative to the norms (no catastrophic cancellation in d2), every
    # pad-involved distance is >= ~3e5 and exp(-dist/bw) underflows to 0.
    rng = np.random.default_rng(987654321)
    sgn = (rng.integers(0, 2, size=(R, D)).astype(np.float32) * 2.0 - 1.0)
    off = 0
    padidx = 0
    padrow = np.zeros(R, bool)
    for c in range(C):
        idx = perm[off:off + counts[c]]
        sx_pad[c * CAP:c * CAP + counts[c]] = src_x[idx]
        W[c * CAP:c * CAP + counts[c], c] = 1.0
        padrow[c * CAP + counts[c]:(c + 1) * CAP] = True
        for p in range(CAP - counts[c]):
            sx_pad[c * CAP + counts[c] + p, :] = 2.0e4 * sgn[padidx]
            padidx += 1
        off += counts[c]
    # per-row d2 epsilon: pad rows have ~1e11 norms where a 0.5 epsilon
    # vanishes in f32 and accumulation noise could push d2 negative (no
    # clamp on device); a 1e9 floor keeps sqrt safe and only perturbs
    # pad distances, whose exp underflows to 0 regardless
    eps_row = np.where(padrow, 1.0e9, EPS).astype(np.float32)

    # round features to bf16 host-side; norms are computed from the rounded
    # values in f32 so the d2 diagonal cancels to ~0 on device
    sx_bf = sx_pad.astype(bf16)
    tx_bf = tgt_x.astype(bf16)
    sx_rf = sx_bf.astype(np.float32)
    tx_rf = tx_bf.astype(np.float32)

    txT_pack = np.zeros((128, 768), bf16)
    for k in range(2):
        txT_pack[:, k * N:(k + 1) * N] = tx_bf.T[k * 128:(k + 1) * 128, :]
    rtcol3 = np.zeros((128, 3), np.float32)
    for blk in range(3):
        rtcol3[:, blk] = (tx_rf[blk * 128:(blk + 1) * 128] ** 2).sum(1) + EPS
    rtrow = (tx_rf ** 2).sum(1)

    cs = counts.astype(np.float64)
    ct = pt.sum(0).astype(np.float64)
    pss = cs * cs
    ptt = ct * ct

    rden2 = (1.0 / (pss[:, None] + pss[None, :]
                    + 2.0 * cs[:, None] * cs[None, :])).astype(np.float32)
    rdenin = (1.0 / (pss + ptt + 2.0 * cs * ct)).astype(np.float32).reshape(C, 1)

    pw5 = np.array([-(float(MU) ** (k - KN // 2)) for k in range(KN)],
                   np.float32)
    pw60 = np.zeros((C, 60), np.float32)
    for k in range(KN):
        pw60[:, k * 12:(k + 1) * 12] = pw5[k]

    in_maps = []
    for r in range(NCORES):
        g = r % 6
        a, b = 2 * g, 2 * g + 1
        pp_active = r < 6
        roll = 2 * g * CAP

        sxr_bf = np.roll(sx_bf, -roll, axis=0)
        sxr_rf = np.roll(sx_rf, -roll, axis=0)
        sxT_pack = np.zeros((128, 1536), bf16)
        for k in range(2):
            sxT_pack[:, k * R:(k + 1) * R] = sxr_bf.T[k * 128:(k + 1) * 128, :]
        norms = (sxr_rf ** 2).sum(1)
        eps_r = np.roll(eps_row, -roll)
        rscol6 = (norms + eps_r).reshape(6, 128).T
        rsrow = norms

        wr = np.roll(W, -roll, axis=0)

        oh2 = np.zeros((C, 2), np.float32)
        oh2[a, 0] = 1.0
        oh2[b, 1] = 1.0

        # k2 split: q0 = class r with all 5 bandwidths; q1 = class 8+(r%4)
        # with bandwidths {0,1,2} on cores 0-3 and {3,4,dup} on cores 4-7.
        c_q0 = r
        c_q1 = 8 + (r % 4)
        kq1 = [0, 1, 2] if r < 4 else [3, 4]
        k2sel = np.zeros((C, 2), np.float32)
        k2sel[c_q0, 0] = 1.0
        k2sel[c_q1, 1] = 1.0
        pw10 = np.zeros((C, 10), np.float32)
        pw10[:, 0:5] = pw5[None, :]
        for j in range(5):
            pw10[:, 5 + j] = pw5[kq1[j]] if j < len(kq1) else pw5[0]

        ptrow2 = np.zeros((2, N), np.float32)
        ptcolf = np.zeros((128, 6), np.float32)
        for q, c in enumerate((c_q0, c_q1)):
            ptrow2[q] = pt[:, c]
            for blk in range(3):
                ptcolf[:, q * 3 + blk] = pt[blk * 128:(blk + 1) * 128, c]

        # perm65: sclT3[j] = ibg[cls, perm(j)] via matmul(lhsT=perm65, rhs=selcol)
        # row layout j = k*12 + t; source col = k*12 + rot(t), rot(t)=(2g+t)%12
        perm65 = np.zeros((65, 65), np.float32)
        for k in range(KN):
            for t in range(12):
                perm65[k * 12 + ((2 * g + t) % 12), k * 12 + t] = 1.0
        for j in range(60, 65):
            perm65[j, j] = 1.0

        wm = np.zeros((128, NCOL), np.float32)
        if pp_active:
            for h, cls in ((0, a), (1, b)):
                for k in range(KN):
                    for t in range(12):
                        if t != cls:
                            wm[h * 64 + k * 12 + t, 0] = I2 / pss[cls]
                        rt_ = (2 * g + t) % 12
                        if rt_ != cls:
                            wm[h * 64 + k * 12 + t, 1] = \
                                -I2 / (cs[cls] * cs[rt_])
                for k in range(KN):
                    wm[h * CAP:(h + 1) * CAP, 2 + k] = 1.0 / (C * pss[cls])
                    wm[h * CAP:(h + 1) * CAP, 7 + k] = \
                        -2.0 / (C * cs[cls] * ct[cls])
        wm[:, 12:17] = 1.0 / (C * ptt[c_q0])
        for j in range(len(kq1)):
            wm[:, 17 + j] = 1.0 / (C * ptt[c_q1])

        ssel = np.zeros((NCOL, 2), np.float32)
        ssel[2:NCOL, 0] = 1.0   # intra cols: k1, k3, k2
        ssel[0:2, 1] = 1.0      # inter cols: T1, T3

        offs = np.zeros((1, 2), np.float32)
        if r == 0:
            corr = 5.0 * CAP - DIAG5 * cs
            offs[0, 0] = -(corr / pss / C).sum()
            offs[0, 1] = -((C - 1) * corr * I2 / pss).sum()

        misc = np.zeros((128, MISCW), np.float32)
        misc[:, O_WM:O_WM + NCOL] = wm
        misc[:, O_ONES] = 1.0
        misc[0, O_ONESR:O_ONESR + 128] = 1.0
        misc[:, O_RSCOL6:O_RSCOL6 + 6] = rscol6
        misc[:, O_RTCOL:O_RTCOL + 3] = rtcol3
        misc[0, O_RSROW:O_RSROW + R] = rsrow
        misc[0, O_RTROW:O_RTROW + N] = rtrow
        misc[0:12, O_EYE12:O_EYE12 + 12] = np.eye(C, dtype=np.float32)
        misc[0:12, O_OH2:O_OH2 + 2] = oh2
        misc[0:12, O_K2SEL:O_K2SEL + 2] = k2sel
        misc[0:65, O_PERM65:O_PERM65 + 65] = perm65
        misc[0:12, O_PW60:O_PW60 + 60] = pw60
        misc[0:12, O_PW10:O_PW10 + 10] = pw10
        misc[0:12, O_RDEN2:O_RDEN2 + 12] = rden2
        misc[0:12, O_RDENIN:O_RDENIN + 1] = rdenin
        misc[0:NCOL, O_SSEL:O_SSEL + 2] = ssel
        misc[0, O_OFFS:O_OFFS + 2] = offs[0]

        miscb = np.zeros((128, MISCBW), np.float32)
        for m in range(6):
            miscb[:, B_WR + m * 12:B_WR + (m + 1) * 12] = \
                wr[m * 128:(m + 1) * 128]
        for m in range(3):
            miscb[:, B_PTB + m * 12:B_PTB + (m + 1) * 12] = \
                pt[m * 128:(m + 1) * 128]
        miscb[0:CAP, B_DIAGM:B_DIAGM + CAP] = 1.0 - np.eye(CAP)
        miscb[CAP:128, B_DIAGM:B_DIAGM + CAP] = 1.0 - np.eye(CAP)
        miscb[:, B_PCF:B_PCF + 6] = ptcolf
        miscb[0, B_PTR:B_PTR + N] = ptrow2[0]
        miscb[0, B_PTR + N:B_PTR + 2 * N] = ptrow2[1]
        miscb[0, B_PTR + 2 * N:B_PTR + 3 * N] = pt[:, a]
        miscb[0, B_PTR + 3 * N:B_PTR + 4 * N] = pt[:, b]

        in_maps.append({
            "sxT": sxT_pack,
            "txT": txT_pack,
            "misc": np.ascontiguousarray(misc),
            "miscb": np.ascontiguousarray(miscb.astype(bf16)),
        })
    return in_maps


def _numpy_fallback(src_x, tgt_x, src_y, tgt_y):
    f = np.float32
    src_x = np.asarray(src_x, f)
    tgt_x = np.asarray(tgt_x, f)
    src_y = np.asarray(src_y).astype(np.int64)
    pt = np.asarray(tgt_y, f)
    ps = np.eye(C, dtype=f)[src_y]

    def cdist(a, bb):
        d2 = (a * a).sum(1)[:, None] + (bb * bb).sum(1)[None, :] - 2.0 * (a @ bb.T)
        return np.sqrt(np.maximum(d2, 0.0))

    def kern(dist, g):
        acc = 0.0
        for i in range(KN):
            bw = np.maximum(np.asarray(g) * (MU ** (i - KN // 2)), 1e-5)
            acc = acc + np.exp(-np.clip(dist / bw, 1e-5, 1e5))
        return acc

    E_ss = cdist(src_x, src_x); E_tt = cdist(tgt_x, tgt_x); E_st = cdist(src_x, tgt_x)
    sss = np.einsum('ic,ij,jc->c', ps, E_ss, ps)
    stt = np.einsum('ic,ij,jc->c', pt, E_tt, pt)
    sst = np.einsum('is,ij,jt->st', ps, E_st, pt)
    cs = ps.sum(0); ct = pt.sum(0)
    pss = cs * cs; ptt = ct * ct; pstd = cs * ct
    g_in = (sss + stt + 2 * np.diagonal(sst)) / (pss + ptt + 2 * pstd)
    Pss = ps.T[:, :, None] * ps.T[:, None, :]
    Ptt = pt.T[:, :, None] * pt.T[:, None, :]
    Pst = ps.T[:, :, None] * pt.T[:, None, :]
    k1 = (kern(E_ss[None] * Pss, g_in[:, None, None]) * Pss).sum((-2, -1)) / pss
    k2 = (kern(E_tt[None] * Ptt, g_in[:, None, None]) * Ptt).sum((-2, -1)) / ptt
    k3 = (kern(E_st[None] * Pst, g_in[:, None, None]) * Pst).sum((-2, -1)) / pstd
    intra = (k1 + k2 - 2 * k3).sum() / C
    sst_s = np.einsum('is,ij,jt->st', ps, E_ss, ps)
    g2 = (sss[:, None] + sss[None, :] + 2 * sst_s) / (
        pss[:, None] + pss[None, :] + 2 * cs[:, None] * cs[None, :])
    T1 = np.zeros((C, C), f); T3 = np.zeros((C, C), f)
    for s in range(C):
        ms = ps[:, s].astype(bool)
        for t in range(C):
            mt = ps[:, t].astype(bool)
            T1[s, t] = kern(E_ss[np.ix_(ms, ms)], g2[s, t]).sum() / pss[s]
            T3[s, t] = kern(E_ss[np.ix_(ms, mt)], g2[s, t]).sum() / (cs[s] * cs[t])
    inter = ((2 * T1 - 2 * T3) * (1 - np.eye(C))).sum() / (C * C - C)
    return np.array([intra, inter], np.float32)


# ----------------------------------------------------------------------------
# device program
# ----------------------------------------------------------------------------

def _build_program():
    import concourse.bass as bass
    import concourse.tile as tile
    from concourse import bacc, mybir

    f32 = mybir.dt.float32
    bf = mybir.dt.bfloat16
    AF = mybir.ActivationFunctionType
    OP = mybir.AluOpType

    nc = bacc.Bacc("TRN2", target_bir_lowering=False, debug=False,
                   num_devices=NCORES)

    i_sxT = nc.dram_tensor("sxT", [128, 2 * R], bf, kind="ExternalInput").ap()
    i_txT = nc.dram_tensor("txT", [128, 2 * N], bf, kind="ExternalInput").ap()
    i_misc = nc.dram_tensor("misc", [128, MISCW], f32, kind="ExternalInput").ap()
    i_miscb = nc.dram_tensor("miscb", [128, MISCBW], bf,
                             kind="ExternalInput").ap()

    o_out = nc.dram_tensor("out", [1, 2], f32, kind="ExternalOutput").ap()

    with tile.TileContext(nc) as tc:
        with (
            tc.tile_pool(name="io", bufs=1) as io,
            tc.tile_pool(name="big", bufs=1) as big,
            tc.tile_pool(name="scr", bufs=2) as scr,
            tc.tile_pool(name="sm", bufs=1) as sm,
            tc.tile_pool(name="pG", bufs=2, space="PSUM") as pG,
            tc.tile_pool(name="p1", bufs=2, space="PSUM") as p1,
            tc.tile_pool(name="pT", bufs=2, space="PSUM") as pT,
            tc.tile_pool(name="pS", bufs=1, space="PSUM") as pS,
            tc.tile_pool(name="dram", bufs=1, space="DRAM") as dpool,
        ):
            dma_sp = nc.sync.dma_start
            dma_act = nc.scalar.dma_start

            # ---------------- input loads: 4 big DMAs ----------------
            sxT = io.tile([128, 2 * R], bf, tag="sxT", name="sxT")
            dma_sp(out=sxT[:], in_=i_sxT[:])
            txT = io.tile([128, 2 * N], bf, tag="txT", name="txT")
            dma_act(out=txT[:], in_=i_txT[:])
            misc = io.tile([128, MISCW], f32, tag="misc", name="misc")
            dma_sp(out=misc[:], in_=i_misc[:])
            miscb = io.tile([128, MISCBW], bf, tag="miscb", name="miscb")
            dma_act(out=miscb[:], in_=i_miscb[:])

            wm = misc[:, O_WM:O_WM + NCOL]
            ones = misc[:, O_ONES:O_ONES + 1]
            onesr = misc[0:1, O_ONESR:O_ONESR + 128]
            rscol6 = misc[:, O_RSCOL6:O_RSCOL6 + 6]
            rtcol = misc[:, O_RTCOL:O_RTCOL + 3]
            rsrow = misc[0:1, O_RSROW:O_RSROW + R]
            rtrow = misc[0:1, O_RTROW:O_RTROW + N]
            eye12 = misc[0:12, O_EYE12:O_EYE12 + 12]
            oh2 = misc[0:12, O_OH2:O_OH2 + 2]
            k2sel = misc[0:12, O_K2SEL:O_K2SEL + 2]
            perm65 = misc[0:65, O_PERM65:O_PERM65 + 65]
            pw60 = misc[0:12, O_PW60:O_PW60 + 60]
            pw10 = misc[0:12, O_PW10:O_PW10 + 10]
            rden2 = misc[0:12, O_RDEN2:O_RDEN2 + 12]
            rdenin = misc[0:12, O_RDENIN:O_RDENIN + 1]
            ssel = misc[0:NCOL, O_SSEL:O_SSEL + 2]
            offs = misc[0:1, O_OFFS:O_OFFS + 2]

            wrb = [miscb[:, B_WR + m * 12:B_WR + (m + 1) * 12] for m in range(6)]
            ptb = [miscb[:, B_PTB + m * 12:B_PTB + (m + 1) * 12]
                   for m in range(3)]
            diagm = miscb[:, B_DIAGM:B_DIAGM + CAP]
            pcf = miscb[:, B_PCF:B_PCF + 6]
            ptr2 = [miscb[0:1, B_PTR + q * N:B_PTR + (q + 1) * N]
                    for q in range(2)]
            ptr3 = [miscb[0:1, B_PTR + (q + 2) * N:B_PTR + (q + 3) * N]
                    for q in range(2)]

            # row-norm broadcasts only (the rest come after the E chain so
            # they can't head-of-line block the Vector queue)
            rsrowb = big.tile([128, R], f32, tag="rsrowb", name="rsrowb")
            rtrowb = big.tile([128, N], f32, tag="rtrowb", name="rtrowb")
            nc.gpsimd.partition_broadcast(rsrowb[:], rsrow)
            nc.gpsimd.partition_broadcast(rtrowb[:], rtrow)

            # ---------------- E matrices (bf16 in/out, f32 d2) ----------------
            sxTk = [sxT[:, 0:R], sxT[:, R:2 * R]]
            txTk = [txT[:, 0:N], txT[:, N:2 * N]]

            def emit_E(dst, lhsT_k, lhs_lo, rhs_k, n_cols, rcol_ap, rowb):
                done = 0
                while done < n_cols:
                    nchunk = min(512, n_cols - done)
                    gp = pG.tile([128, 512], f32, tag="G", name="gp")
                    for k in range(2):
                        nc.tensor.matmul(
                            gp[:, :nchunk],
                            lhsT_k[k][:, lhs_lo:lhs_lo + 128],
                            rhs_k[k][:, done:done + nchunk],
                            start=(k == 0), stop=(k == 1))
                    t1_ = scr.tile([128, 512], f32, tag="d2scr", name="d2s")
                    nc.vector.scalar_tensor_tensor(
                        out=t1_[:, :nchunk], in0=gp[:, :nchunk], scalar=-2.0,
                        in1=rowb[:, done:done + nchunk],
                        op0=OP.mult, op1=OP.add)
                    nc.scalar.activation(dst[:, done:done + nchunk],
                                         t1_[:, :nchunk], AF.Sqrt,
                                         bias=rcol_ap)
                    done += nchunk

            # own slab first (feeds the T1/T3 gathers), then E_tt (k2 builds),
            # then E_pt slab 0 (k3), then the remaining slabs for the sums
            E_ss = [big.tile([128, R], bf, tag=f"E_ss{s}", name=f"E_ss{s}")
                    for s in range(6)]
            E_own = E_ss[0]
            emit_E(E_own, sxTk, 0, sxTk, R, rscol6[:, 0:1], rsrowb)

            E_ttf = big.tile([128, 3 * N], bf, tag="E_ttf", name="E_ttf")
            for blk in range(3):
                emit_E(E_ttf[:, blk * N:(blk + 1) * N], txTk, blk * 128, txTk,
                       N, rtcol[:, blk:blk + 1], rtrowb)

            E_ptf = big.tile([128, 6 * N], bf, tag="E_ptf", name="E_ptf")
            for s in range(6):
                emit_E(E_ptf[:, s * N:(s + 1) * N], sxTk, s * 128, txTk, N,
                       rscol6[:, s:s + 1], rtrowb)

            for s in range(1, 6):
                emit_E(E_ss[s], sxTk, s * 128, sxTk, R, rscol6[:, s:s + 1],
                       rsrowb)

            # ---------------- local global sums (no collective) --------------
            part = sm.tile([C, 36], f32, tag="part", name="part")

            s1ps = pS.tile([C, C], f32, tag="S", name="s1ps")
            for sub in range(6):
                pp = p1.tile([128, C], f32, tag="p1", name="pp")
                for slab in range(6):
                    nc.tensor.matmul(
                        pp[:], E_ss[slab][:, sub * 128:(sub + 1) * 128],
                        wrb[slab], start=(slab == 0), stop=(slab == 5))
                cb = scr.tile([128, C], bf, tag="cbs", name="cb")
                nc.vector.tensor_copy(cb[:], pp[:])
                nc.tensor.matmul(s1ps[:], cb[:], wrb[sub],
                                 start=(sub == 0), stop=(sub == 5))
            nc.vector.tensor_copy(part[:, 0:12], s1ps[:])

            stps = pS.tile([C, C], f32, tag="S", name="stps")
            for sub in range(3):
                pp = p1.tile([128, C], f32, tag="p1", name="pp")
                for blk in range(3):
                    nc.tensor.matmul(
                        pp[:],
                        E_ttf[:, blk * N + sub * 128:blk * N + (sub + 1) * 128],
                        ptb[blk], start=(blk == 0), stop=(blk == 2))
                cb = scr.tile([128, C], bf, tag="cbs", name="cb")
                nc.vector.tensor_copy(cb[:], pp[:])
                nc.tensor.matmul(stps[:], cb[:], ptb[sub],
                                 start=(sub == 0), stop=(sub == 2))
            nc.vector.tensor_copy(part[:, 12:24], stps[:])

            ssps = pS.tile([C, C], f32, tag="S", name="ssps")
            for sub in range(3):
                pp = p1.tile([128, C], f32, tag="p1", name="pp")
                for slab in range(6):
                    nc.tensor.matmul(
                        pp[:],
                        E_ptf[:, slab * N + sub * 128:slab * N + (sub + 1) * 128],
                        wrb[slab], start=(slab == 0), stop=(slab == 5))
                cb = scr.tile([128, C], bf, tag="cbs", name="cb")
                nc.vector.tensor_copy(cb[:], pp[:])
                nc.tensor.matmul(ssps[:], cb[:], ptb[sub],
                                 start=(sub == 0), stop=(sub == 2))
            nc.vector.tensor_copy(part[:, 24:36], ssps[:])

            # ---------------- T1/T3 gathers + k2/k3 builds ----------------
            E_diag = big.tile([128, CAP], bf, tag="E_diag", name="E_diag")
            nc.vector.tensor_tensor(E_diag[0:CAP, :], E_own[0:CAP, 0:CAP],
                                    diagm[0:CAP, :], OP.mult)
            nc.vector.tensor_tensor(E_diag[CAP:128, :],
                                    E_own[CAP:128, CAP:128],
                                    diagm[CAP:128, :], OP.mult)

            d_ed = dpool.tile([128, CAP], bf, tag="d_ed", name="d_ed")
            dma_sp(out=d_ed[:], in_=E_diag[:])
            # E_own -> DRAM in class-block layout [t][row][col] so each
            # (half, t) block is one contiguous segment
            d_eob = dpool.tile([C, 128 * CAP], bf, tag="d_eob", name="d_eob")
            dst_eob = bass.AP(tensor=d_eob.tensor, offset=0,
                              ap=[[CAP, 128], [128 * CAP, 12], [1, CAP]])
            dma_act(out=dst_eob, in_=E_own[:])

            t1src = big.tile([128, CAP * CAP], bf, tag="t1src", name="t1src")
            for h in range(2):
                ap_in = bass.AP(tensor=d_ed.tensor, offset=h * CAP * CAP,
                                ap=[[0, 64], [1, CAP * CAP]])
                dma_sp(out=t1src[h * 64:(h + 1) * 64, :], in_=ap_in)

            # T3 flat blocks: partition h*64 + k*12 + t <- block (h, t)
            t3src = big.tile([128, CAP * CAP], bf, tag="t3src", name="t3src")
            for h in range(2):
                ap_in = bass.AP(tensor=d_eob.tensor, offset=h * CAP * CAP,
                                ap=[[0, 5], [128 * CAP, 12], [1, CAP * CAP]])
                dma_act(out=t3src[h * 64:h * 64 + 60, :], in_=ap_in)
                ap_pad = bass.AP(tensor=d_eob.tensor, offset=h * CAP * CAP,
                                 ap=[[0, 4], [1, CAP * CAP]])
                dma_act(out=t3src[h * 64 + 60:h * 64 + 64, :], in_=ap_pad)

            # late gpsimd broadcasts (after the E-chain Vector work)
            ptrow2b = [big.tile([128, N], bf, tag=f"ptrow2b{q}",
                                name=f"ptrow2b{q}") for q in range(2)]
            nc.gpsimd.partition_broadcast(ptrow2b[0][:], ptr2[0])
            nc.gpsimd.partition_broadcast(ptrow2b[1][:], ptr2[1])
            ptw3 = big.tile([128, N], bf, tag="ptw3", name="ptw3")
            ptw3t = big.tile([128, N], bf, tag="ptw3t", name="ptw3t")
            nc.gpsimd.partition_broadcast(ptw3[:], ptr3[0])
            nc.gpsimd.partition_broadcast(ptw3t[:], ptr3[1])
            nc.vector.tensor_copy(ptw3[CAP:128, :], ptw3t[CAP:128, :])

            k2P = []
            k2D = []
            for q in range(2):
                P = big.tile([128, 3 * N], bf, tag=f"k2P{q}", name=f"k2P{q}")
                colap = bass.AP(tensor=pcf.tensor,
                                offset=pcf.offset + q * 3,
                                ap=[list(pcf.ap[0]), [1, 3], [0, N]])
                rowap = bass.AP(tensor=ptrow2b[q].tensor,
                                offset=ptrow2b[q].offset,
                                ap=[list(ptrow2b[q].ap[0]), [0, 3], [1, N]])
                nc.vector.tensor_tensor(P[:], colap, rowap, OP.mult)
                Dt = big.tile([128, 3 * N], bf, tag=f"k2D{q}", name=f"k2D{q}")
                nc.vector.tensor_tensor(Dt[:], E_ttf[:], P[:], OP.mult)
                k2P.append(P)
                k2D.append(Dt)

            k3D = big.tile([128, N], bf, tag="k3D", name="k3D")
            nc.vector.tensor_tensor(k3D[:], E_ptf[:, 0:N], ptw3[:], OP.mult)

            acc = big.tile([128, NCOL], f32, tag="acc", name="acc")
            nc.vector.memset(acc[:], 0.0)
            sclT1 = sm.tile([128, 1], f32, tag="sclT1", name="sclT1")
            sclT3 = sm.tile([128, 1], f32, tag="sclT3", name="sclT3")
            nc.vector.memset(sclT1[:], 0.0)
            nc.vector.memset(sclT3[:], 0.0)

            # ---------------- gammas (from the local sums) ----------------
            S1 = part[:, 0:12]
            sttM = part[:, 12:24]
            sstM = part[:, 24:36]

            def diag_col(mat, nm):
                s_ = scr.tile([C, C], f32, tag="diagscr", name="dsc")
                col = sm.tile([C, 1], f32, tag=nm, name=nm)
                nc.vector.tensor_tensor(s_[:], mat, eye12, OP.mult)
                nc.vector.reduce_sum(out=col[:], in_=s_[:],
                                     axis=mybir.AxisListType.X)
                return col

            ssscol = diag_col(S1, "ssscol")
            sttcol = diag_col(sttM, "sttcol")
            sstdcol = diag_col(sstM, "sstdcol")

            gin = sm.tile([C, 1], f32, tag="gin", name="gin")
            nc.vector.scalar_tensor_tensor(out=gin[:], in0=sstdcol[:], scalar=2.0,
                                           in1=sttcol[:], op0=OP.mult, op1=OP.add)
            nc.vector.tensor_tensor(gin[:], gin[:], ssscol[:], OP.add)
            nc.vector.tensor_tensor(gin[:], gin[:], rdenin, OP.mult)

            ssst = pT.tile([1, C], f32, tag="tiny", name="ssst")
            nc.tensor.transpose(ssst[:], ssscol[:], eye12)
            ssstsb = sm.tile([1, C], f32, tag="ssstsb", name="ssstsb")
            nc.vector.tensor_copy(ssstsb[:], ssst[:])
            ps12 = pT.tile([C, C], f32, tag="tiny", name="ps12")
            nc.tensor.matmul(ps12[:], onesr[0:1, 0:12], ssstsb[:],
                             start=True, stop=True)
            sssrowb = sm.tile([C, C], f32, tag="sssrowb", name="sssrowb")
            nc.vector.tensor_copy(sssrowb[:], ps12[:])
            g2 = sm.tile([C, C], f32, tag="g2", name="g2")
            nc.vector.tensor_scalar(g2[:], S1, 2.0, None, OP.mult)
            nc.vector.tensor_tensor(g2[:], g2[:], sssrowb[:], OP.add)
            nc.vector.tensor_scalar(g2[:], g2[:], ssscol[:], None, OP.add)
            nc.vector.tensor_tensor(g2[:], g2[:], rden2, OP.mult)

            # IBG [12, 70] = -1/bw : cols 0-59 from g2 (k-major),
            # 60-64 from gin (k-order), 65-69 from gin (host-permuted for q1)
            ibg0 = sm.tile([C, 70], f32, tag="ibg0", name="ibg0")
            g2ap = g2[:]
            g2exp = bass.AP(tensor=g2ap.tensor, offset=g2ap.offset,
                            ap=[list(g2ap.ap[0]), [0, 5], [1, 12]])
            nc.vector.tensor_tensor(ibg0[:, 0:60], g2exp, pw60, OP.mult)
            ginap = gin[:]
            ginexp = bass.AP(tensor=ginap.tensor, offset=ginap.offset,
                             ap=[list(ginap.ap[0]), [0, 10]])
            nc.vector.tensor_tensor(ibg0[:, 60:70], ginexp, pw10, OP.mult)
            nc.vector.tensor_scalar(ibg0[:], ibg0[:], -1e-5, None, OP.min)
            ibg = sm.tile([C, 70], f32, tag="ibg", name="ibg")
            nc.vector.reciprocal(ibg[:], ibg0[:])

            # per-half scale vectors
            negk1 = sm.tile([128, 5], f32, tag="negk1", name="negk1")
            for h in range(2):
                ps_ = pT.tile([1, 65], f32, tag="tiny", name="psel")
                nc.tensor.matmul(ps_[:], oh2[:, h:h + 1], ibg[:, 0:65],
                                 start=True, stop=True)
                selsb = sm.tile([1, 65], f32, tag=f"sel{h}", name=f"sel{h}")
                nc.vector.tensor_copy(selsb[:], ps_[:])
                tp_ = pT.tile([65, 1], f32, tag="tiny", name="tsel")
                nc.tensor.transpose(tp_[:], selsb[:], ones[0:1, :])
                tpsb = scr.tile([65, 1], f32, tag="tselsb", name="tpsb")
                nc.vector.tensor_copy(tpsb[:], tp_[:])
                nc.vector.tensor_copy(sclT1[h * 64:h * 64 + 60, :], tpsb[0:60, :])
                p3 = pT.tile([65, 1], f32, tag="tiny", name="p3")
                nc.tensor.matmul(p3[:], perm65, tpsb[:], start=True, stop=True)
                p3sb = scr.tile([65, 1], f32, tag="p3sb", name="p3sb")
                nc.vector.tensor_copy(p3sb[:], p3[:])
                nc.vector.tensor_copy(sclT3[h * 64:h * 64 + 60, :], p3sb[0:60, :])
                pnk = pT.tile([128, 5], f32, tag="tiny", name="pnk")
                nc.tensor.matmul(pnk[:], onesr, selsb[0:1, 60:65],
                                 start=True, stop=True)
                if h == 0:
                    nc.vector.tensor_copy(negk1[0:CAP, :], pnk[0:CAP, :])
                else:
                    nc.vector.tensor_copy(negk1[CAP:128, :], pnk[CAP:128, :])

            negb = []
            for q in range(2):
                k2sc = pT.tile([1, 5], f32, tag="tiny", name="k2sc")
                nc.tensor.matmul(k2sc[:], k2sel[:, q:q + 1],
                                 ibg[:, 60 + 5 * q:65 + 5 * q],
                                 start=True, stop=True)
                k2scsb = sm.tile([1, 5], f32, tag=f"k2scsb{q}", name=f"k2scsb{q}")
                nc.vector.tensor_copy(k2scsb[:], k2sc[:])
                pnb = pT.tile([128, 5], f32, tag="tiny", name="pnb")
                nc.tensor.matmul(pnb[:], onesr, k2scsb[:], start=True, stop=True)
                nb = sm.tile([128, 5], f32, tag=f"negb{q}", name=f"negb{q}")
                nc.vector.tensor_copy(nb[:], pnb[:])
                negb.append(nb)

            # ---------------- exp passes ----------------
            nc.scalar.activation(t1src[:], t1src[:], AF.Exp, scale=sclT1[:],
                                 accum_out=acc[:, 0:1])
            nc.scalar.activation(t3src[:], t3src[:], AF.Exp, scale=sclT3[:],
                                 accum_out=acc[:, 1:2])

            for k in range(KN):
                sk = scr.tile([128, CAP], bf, tag="k1scr", name="sk1")
                nc.scalar.activation(sk[:], E_diag[:], AF.Exp,
                                     scale=negk1[:, k:k + 1],
                                     accum_out=acc[:, 2 + k:3 + k])

            for k in range(KN):
                ek = scr.tile([128, N], bf, tag="k3e", name="ek3")
                nc.scalar.activation(ek[:], k3D[:], AF.Exp,
                                     scale=negk1[:, k:k + 1])
                sk = scr.tile([128, N], bf, tag="k3scr", name="sk3")
                nc.vector.scalar_tensor_tensor(
                    out=sk[:], in0=ek[:], scalar=1.0, in1=ptw3[:],
                    op0=OP.mult, op1=OP.mult,
                    accum_out=acc[:, 7 + k:8 + k])

            for q in range(2):
                npass = 5 if q == 0 else 3
                for j in range(npass):
                    ek = scr.tile([128, 3 * N], bf, tag="k2e", name="ek2")
                    nc.scalar.activation(ek[:], k2D[q][:], AF.Exp,
                                         scale=negb[q][:, j:j + 1])
                    sk = scr.tile([128, 3 * N], bf, tag="k2scr", name="sk2")
                    col = 12 + 5 * q + j
                    nc.vector.scalar_tensor_tensor(
                        out=sk[:], in0=ek[:], scalar=1.0, in1=k2P[q][:],
                        op0=OP.mult, op1=OP.mult,
                        accum_out=acc[:, col:col + 1])

            # ---------------- final weighted reduce ----------------
            v = big.tile([128, NCOL], f32, tag="v", name="v")
            nc.vector.tensor_tensor(v[:], acc[:], wm, OP.mult)
            m1 = pT.tile([NCOL, 1], f32, tag="tiny", name="m1")
            nc.tensor.matmul(m1[:], v[:], ones, start=True, stop=True)
            m1sb = sm.tile([NCOL, 1], f32, tag="m1sb", name="m1sb")
            nc.vector.tensor_copy(m1sb[:], m1[:])
            m2 = pT.tile([1, 2], f32, tag="tiny", name="m2")
            nc.tensor.matmul(m2[:], m1sb[:], ssel, start=True, stop=True)
            res = sm.tile([1, 2], f32, tag="res", name="res")
            nc.vector.tensor_tensor(res[:], m2[:], offs, OP.add)
            dma_sp(out=o_out[:], in_=res[:])

    nc.compile()
    return nc


def get_program():
    if "nc" not in _COMPILED:
        _COMPILED["nc"] = _build_program()
    return _COMPILED["nc"]


# ----------------------------------------------------------------------------
# entry point
# ----------------------------------------------------------------------------

def _run(in_maps, trace=False):
    from concourse.bass_utils import run_bass_kernel_spmd
    nc = get_program()
    return run_bass_kernel_spmd(nc, in_maps, list(range(NCORES)), trace=trace)


def kernel(src_x, tgt_x, src_y, tgt_y):
    in_maps = _host_prep(src_x, tgt_x, src_y, tgt_y)
    if in_maps is None:
        return _numpy_fallback(src_x, tgt_x, src_y, tgt_y)
    br = _run(in_maps)
    total = np.zeros(2, np.float64)
    for res in br.results:
        total += res["out"].reshape(2).astype(np.float64)
    return total.astype(np.float32)
